# revision 1
# baseline (speedup 1.0000x reference)
"""Trainium2 Bass kernel for the SLAYER-style 2-layer spiking MLP.

Reference computation (per batch element n):
    flat   = input.reshape(64, 3072)
    a1     = flat @ w1.T                      (constant over time)
    u1[t]  = a1 * c[t]          where c = cumsum(srm kernel)  (PSP of a
             time-constant input is just a ramp scale)
    s1     = spike_scan(u1)     sequential threshold w/ refractory feedback
    a2[t]  = w2 @ s1[:, t]
    u2     = psp(a2)            (true temporal conv, srm kernel)
    out    = spike_scan(u2)

Key algebraic facts exploited on-device:
  * The refractory kernel rk[d] = -20*d*e^{1-d} (d=1..32) is
    polynomial-geometric, so the refractory sum r[t] = sum_d rk[d] s[t-d]
    follows an exact order-2 recurrence:
        P[t] = q*P[t-1] + s[t-1]
        R[t] = q*R[t-1] + P[t]          (q = e^-1, states scaled by -1/20)
        spike:  u + (-20)*R >= theta  <=>  R + 0.5 <= u/20
    The d>32 truncation of rk is ~1e-11 and far below fp32 noise.
  * Each scan step is exactly 3 fused scalar_tensor_tensor DVE ops over a
    [104, 33] tile holding both layers (layer 2 rides along lagged 32 steps).
  * The SRM PSP filter srm[k] = (k/10)e^{1-k/10} is the same confluent
    geometric form, handled by two hardware tensor_tensor_scan ops per
    16-step block (never truncated within T=100, so it is exact).

Sharding: data-parallel over batch, 8 elements per core, weights replicated.
"""

import numpy as np

NB = 8            # batch elements per core
T = 100           # timesteps
B = 16            # pipeline block size
LAG = 32          # layer-2 ride-along lag (>= refractory window 32)
TF = T + LAG      # fused scan steps
NCOL = 33         # 32 layer-1 columns (4 chunks x 8 batch) + 1 layer-2 column
PMAX = 104        # padded partition count per o-chunk
MC = [103, 103, 102, 102]      # o-chunk sizes (sum = 410)
OFF = [0, 103, 206, 308]
KT = 24           # 3072 / 128 k-tiles
NO1 = 410
NO2 = 10

_CACHE = {}


def _consts():
    q = float(np.float32(np.exp(-1.0)))          # refractory ratio
    p = float(np.float32(np.exp(-0.1)))          # SRM ratio
    k2 = float(np.float32(np.exp(1.0) / 200.0))  # a2 pre-scale: u2/20 = sum
    return q, p, k2


def build():
    import concourse.bass as bass
    import concourse.bacc as bacc
    import concourse.mybir as mybir
    from concourse import tile

    f32 = mybir.dt.float32
    Alu = mybir.AluOpType
    q, p, k2 = _consts()

    nc = bacc.Bacc("TRN2", target_bir_lowering=False, debug=False, num_devices=8)

    flatT_d = nc.dram_tensor("flatT", [KT * 128, NB], f32, kind="ExternalInput")
    w1T_d = nc.dram_tensor("w1T", [KT * 128, NO1], f32, kind="ExternalInput")
    w2p_d = nc.dram_tensor("w2p", [PMAX, 4, NO2], f32, kind="ExternalInput")
    c20_d = nc.dram_tensor("c20rep", [PMAX, T], f32, kind="ExternalInput")
    pc_d = nc.dram_tensor("pconst", [80, B], f32, kind="ExternalInput")
    sel_d = nc.dram_tensor("sel", [NO2, NB, 80], f32, kind="ExternalInput")
    eye_d = nc.dram_tensor("eye8", [NB, NB], f32, kind="ExternalInput")
    out_d = nc.dram_tensor("out", [80, T], f32, kind="ExternalOutput")

    with tile.TileContext(nc) as tc:
        with (
            tc.tile_pool(name="pers", bufs=1) as pool,
            tc.tile_pool(name="ps1", bufs=1, space="PSUM") as ps1,
            tc.tile_pool(name="ps2", bufs=2, space="PSUM") as ps2,
        ):
            w1sb = pool.tile([128, KT, NO1], f32, tag="w1sb")
            fTsb = pool.tile([128, KT, NB], f32, tag="fTsb")
            w2sb = pool.tile([PMAX, 4, NO2], f32, tag="w2sb")
            c20sb = pool.tile([PMAX, T], f32, tag="c20sb")
            pcsb = pool.tile([80, B], f32, tag="pcsb")
            A1 = pool.tile([PMAX, 32], f32, tag="A1")
            Up = pool.tile([PMAX, TF, NCOL], f32, tag="Up")
            # mega-tile: spike history S (TF+1 slots of NCOL) followed by the
            # interleaved IIR state [P(NCOL) | R(NCOL)] — one address space so
            # a single dual-range AP can feed (s_tau | P) to the merged
            # state-update op.
            SW = (TF + 1) * NCOL
            M = pool.tile([PMAX, SW + 2 * NCOL], f32, tag="M")
            selsb = pool.tile([NO2, NB, 80], f32, tag="selsb")
            eyesb = pool.tile([NB, NB], f32, tag="eyesb")
            a1rsb = pool.tile([NB, NO1], f32, tag="a1rsb")
            a2tmp = pool.tile([NO2, B, NB], f32, tag="a2tmp")
            a2s = pool.tile([80, T + 1], f32, tag="a2s")
            W1 = pool.tile([80, T + 1], f32, tag="W1")
            W2 = pool.tile([80, T + 1], f32, tag="W2")
            ostage = pool.tile([80, T], f32, tag="ostage")

            # ---- input DMAs (small tensors first: the stationary matmul
            # operand and constants gate everything else) ----
            nc.sync.dma_start(
                fTsb[:], flatT_d[:].rearrange("(k p) n -> p k n", p=128)
            )
            nc.sync.dma_start(w2sb[:], w2p_d[:])
            nc.sync.dma_start(c20sb[:], c20_d[:])
            nc.sync.dma_start(pcsb[:], pc_d[:])
            nc.sync.dma_start(selsb[:], sel_d[:])
            nc.sync.dma_start(eyesb[:], eye_d[:])
            for k in range(KT):
                for h in range(2):
                    r0, r1 = 64 * h, 64 * (h + 1)
                    nc.sync.dma_start(
                        w1sb[r0:r1, k, :],
                        w1T_d[k * 128 + r0:k * 128 + r1, :],
                    )

            # ---- state init ----
            nc.gpsimd.memset(Up[:], 0.0)
            nc.vector.memset(A1[:], 0.0)
            nc.vector.memset(M[:, 0:NCOL], 0.0)            # S slot 0
            nc.vector.memset(M[:, SW:SW + 2 * NCOL], 0.0)  # P | R
            nc.vector.memset(a2s[:, 0:1], 0.0)
            nc.vector.memset(W1[:, 0:1], 0.0)
            nc.vector.memset(W2[:, 0:1], 0.0)

            # ---- fc1: A1row[n, o] = flat @ w1.T, accumulated over k.
            # flatT tiles are already [K, 8] so they serve as the (tiny)
            # stationary operand; w1T tiles stream as the moving operand.
            a1row = ps1.tile([NB, NO1], f32, tag="a1row", name="a1row")
            for k in range(KT):
                nc.tensor.matmul(
                    a1row[:],
                    fTsb[:, k, :],
                    w1sb[:, k, :],
                    start=(k == 0),
                    stop=(k == KT - 1),
                )
            nc.scalar.activation(
                a1rsb[:], a1row[:], mybir.ActivationFunctionType.Copy
            )
            # transpose to scan layout: A1[o_chunk, (c, n)]
            for c in range(4):
                a1tp = ps1.tile(
                    [PMAX, NB], f32, tag=f"a1tp{c % 2}", name="a1tp"
                )
                nc.tensor.transpose(
                    a1tp[0:MC[c], :],
                    a1rsb[0:NB, OFF[c]:OFF[c] + MC[c]],
                    eyesb[:],
                )
                nc.scalar.activation(
                    A1[0:MC[c], c * NB:(c + 1) * NB],
                    a1tp[0:MC[c], :],
                    mybir.ActivationFunctionType.Copy,
                )

            # ---- u1/20 for layer-1 columns: Up[:, t, j] = A1[:, j] * c20[t] ----
            c20b = c20sb[:].unsqueeze(2).broadcast_to([PMAX, T, 32])
            a1b = A1[:].unsqueeze(1).broadcast_to([PMAX, T, 32])
            nc.vector.scalar_tensor_tensor(
                Up[:, 0:T, 0:32], c20b, 0.0, a1b, Alu.bypass, Alu.mult
            )

            # ---- fused scan: layer-1 at step tau, layer-2 at tau-LAG ----
            # Emit block-b's psp2 pipeline DELAY steps after its last spike
            # step, so the PE matmuls have slack before DVE needs the result.
            MW = M.ap[0][0]          # mega-tile row stride (elements)
            MOFF = M.offset

            def m_ap(off, dims, parts=PMAX):
                return bass.AP(M.tensor, MOFF + off, [[MW, parts]] + dims)

            st_out = m_ap(SW, [[1, 2 * NCOL]])          # [P | R]
            r_in = m_ap(SW + NCOL, [[1, NCOL]])         # R

            DELAY = 13
            blocks = []
            for b in range((T + B - 1) // B):
                tb0, tb1 = b * B, min((b + 1) * B, T)
                blocks.append((tb0, tb1))
            block_at = {tb1 - 1 + DELAY: (tb0, tb1) for tb0, tb1 in blocks}

            p_st = m_ap(SW, [[1, NCOL]])
            for tau in range(TF):
                if tau < T:
                    # full width: 32 layer-1 columns + layer-2 column
                    pp, rr, w0, off = p_st, r_in, NCOL, 0
                else:
                    # tail: layer-1 finished, only column 32 is live
                    pp = m_ap(SW + 32, [[1, 1]])
                    rr = m_ap(SW + NCOL + 32, [[1, 1]])
                    w0, off = 1, 32
                # P = q*P + s_{tau-1}
                nc.vector.scalar_tensor_tensor(
                    pp, pp, q, m_ap(tau * NCOL + off, [[1, w0]]),
                    Alu.mult, Alu.add,
                )
                # R = q*R + P
                nc.vector.scalar_tensor_tensor(
                    rr, rr, q, pp, Alu.mult, Alu.add,
                )
                # s_{tau} = (R + 0.5) <= u/20
                nc.vector.scalar_tensor_tensor(
                    m_ap((tau + 1) * NCOL + off, [[1, w0]]),
                    rr,
                    0.5,
                    Up[:, tau, off:off + w0],
                    Alu.add,
                    Alu.is_le,
                )

                if tau in block_at:
                    tb0, tb1 = block_at[tau]
                    blk = tb1 - tb0
                    # a2[o2, t, n] for t in [tb0, tb1): 4 chunk-matmuls
                    a2ps = ps2.tile([NO2, B, NB], f32, tag="a2ps", name="a2ps")
                    for c in range(4):
                        nc.tensor.matmul(
                            a2ps[:, 0:blk, :],
                            w2sb[:, c, :],
                            m_ap((tb0 + 1) * NCOL + c * NB,
                                 [[NCOL, blk], [1, NB]]),
                            start=(c == 0),
                            stop=(c == 3),
                        )
                    # evac with pre-scale: a2tmp = a2 * e/200  (still [10,(t,n)])
                    nc.scalar.activation(
                        a2tmp[:, 0:blk, :],
                        a2ps[:, 0:blk, :],
                        mybir.ActivationFunctionType.Copy,
                        scale=k2,
                    )
                    # re-stack to [80, t] (row = n*10+o2) via selector matmuls
                    a2r = ps2.tile([80, B], f32, tag="a2r", name="a2r")
                    for n in range(NB):
                        nc.tensor.matmul(
                            a2r[:, 0:blk],
                            selsb[:, n, :],
                            a2tmp[:, 0:blk, n],
                            start=(n == 0),
                            stop=(n == NB - 1),
                        )
                    nc.scalar.activation(
                        a2s[:, tb0 + 1:tb1 + 1],
                        a2r[:, 0:blk],
                        mybir.ActivationFunctionType.Copy,
                    )
                    # W1[t] = (a2s[t-1] + W1[t-1]) * p   (hardware scan)
                    nc.vector.tensor_tensor_scan(
                        W1[:, tb0 + 1:tb1 + 1],
                        a2s[:, tb0:tb1],
                        pcsb[:, 0:blk],
                        W1[:, tb0:tb0 + 1],
                        Alu.add,
                        Alu.mult,
                    )
                    # W2[t] = (W1[t-1] + W2[t-1]) * p
                    nc.vector.tensor_tensor_scan(
                        W2[:, tb0 + 1:tb1 + 1],
                        W1[:, tb0:tb1],
                        pcsb[:, 0:blk],
                        W2[:, tb0:tb0 + 1],
                        Alu.add,
                        Alu.mult,
                    )
                    # u2/20 = W1 + W2 -> layer-2 column of Up, lagged by LAG
                    nc.gpsimd.tensor_tensor(
                        Up[0:80, tb0 + LAG:tb1 + LAG, 32],
                        W1[:, tb0 + 1:tb1 + 1],
                        W2[:, tb0 + 1:tb1 + 1],
                        Alu.add,
                    )

            # ---- output: layer-2 spikes, fused steps LAG..LAG+T ----
            nc.vector.tensor_copy(
                ostage[:],
                m_ap((LAG + 1) * NCOL + 32, [[NCOL, T]], parts=80),
            )
            nc.sync.dma_start(out_d[:], ostage[:])

    nc.compile()
    return nc


def _host_inputs(input, w1, w2):
    f32 = np.float32
    q, p, k2 = _consts()
    flat = np.ascontiguousarray(input.reshape(64, -1).astype(f32))
    flatT = np.ascontiguousarray(flat.T)                      # (3072, 64)
    w1T = np.ascontiguousarray(w1.astype(f32).T)              # (3072, 410)
    w2p = np.zeros((PMAX, 4, NO2), f32)
    for c in range(4):
        w2p[0:MC[c], c, :] = w2.astype(f32)[:, OFF[c]:OFF[c] + MC[c]].T
    t = np.arange(T, dtype=np.float64)
    srm = (t / 10.0) * np.exp(1.0 - t / 10.0)
    c20 = (np.cumsum(srm) / 20.0).astype(f32)
    c20rep = np.broadcast_to(c20, (PMAX, T)).copy()
    pconst = np.full((80, B), p, f32)
    sel = np.zeros((NO2, NB, 80), f32)
    for n in range(NB):
        for o2 in range(NO2):
            sel[o2, n, n * NO2 + o2] = 1.0
    eye8 = np.eye(NB, dtype=f32)
    return flatT, w1T, w2p, c20rep, pconst, sel, eye8


def kernel(input, w1, w2):
    from concourse.bass_utils import run_bass_kernel_spmd

    if "nc" not in _CACHE:
        _CACHE["nc"] = build()
    nc = _CACHE["nc"]

    flatT, w1T, w2p, c20rep, pconst, sel, eye8 = _host_inputs(input, w1, w2)
    in_maps = []
    for core in range(8):
        in_maps.append({
            "flatT": np.ascontiguousarray(flatT[:, core * NB:(core + 1) * NB]),
            "w1T": w1T,
            "w2p": w2p,
            "c20rep": c20rep,
            "pconst": pconst,
            "sel": sel,
            "eye8": eye8,
        })
    res = run_bass_kernel_spmd(nc, in_maps, core_ids=list(range(8)))
    full = np.zeros((64, NO2, T), np.float32)
    for core in range(8):
        full[core * NB:(core + 1) * NB] = (
            res.results[core]["out"].reshape(NB, NO2, T)
        )
    return full



# revision 12
# speedup vs baseline: 1.0700x; 1.0700x over previous
"""Trainium2 Bass kernel for the SLAYER-style 2-layer spiking MLP.

Reference computation (per batch element n):
    flat   = input.reshape(64, 3072)
    a1     = flat @ w1.T                      (constant over time)
    u1[t]  = a1 * c[t]          where c = cumsum(srm kernel)  (PSP of a
             time-constant input is just a ramp scale)
    s1     = spike_scan(u1)     sequential threshold w/ refractory feedback
    a2[t]  = w2 @ s1[:, t]
    u2     = psp(a2)            (true temporal conv, srm kernel)
    out    = spike_scan(u2)

Refractory feedback is an exact order-2 IIR (kernel rk[d] = -20 d e^{1-d}):
    P[t] = q*P[t-1] + s[t-1];  R[t] = q*R[t-1] + P[t]   (q = e^-1)
    spike:  (R + 0.5) <= u/20
Each fused scan step is 3 scalar_tensor_tensor DVE ops over a [104, 33]
tile holding both layers (layer 2 rides along lagged LAG steps).

Schedule highlights vs the naive version:
  * w1 / flat are host-packed so each DMA moves long contiguous
    per-partition lines (128 descriptors instead of 3072).
  * The u1/20 threshold rows are produced per-step on the otherwise-idle
    Scalar engine (activation Copy, scale=c20[t]) instead of one huge
    broadcast-AP DVE op that serialized the whole prologue.
  * PSUM evacuations ride the scalar stream at delayed slots so they
    never block threshold-row production.
  * fc2 re-stack uses a shared eye(10) stationary with per-n PSUM
    partition-offset writes (1 LDWEIGHTS instead of 8 full selector
    loads per block).

Sharding: data-parallel over batch, 8 elements per core, weights replicated.
"""

import numpy as np

NB = 8            # batch elements per core
T = 100           # timesteps
B = 16            # pipeline block size
LAG = 36          # layer-2 ride-along lag (>= block + pipeline latency)
TF = T + LAG      # fused scan steps
NCOL = 33         # 32 layer-1 columns (4 chunks x 8 batch) + 1 layer-2 column
PMAX = 104        # padded partition count per o-chunk
MC = [103, 103, 102, 102]      # o-chunk sizes (sum = 410)
OFF = [0, 103, 206, 308]
KT = 24           # 3072 / 128 k-tiles
KC = 6            # k-tiles per w1 DMA chunk
NO1 = 410
NO2 = 10
DELAY = 19        # DVE-stream slots after block end before W1/W2 scans
EVAC1_DELAY = 9   # scalar-stream slots after block end before PSUM evac 1
EVAC2_DELAY = 17  # (selector MMs are emitted right after evac1)

_CACHE = {}


def _consts():
    q = float(np.float32(np.exp(-1.0)))          # refractory ratio
    p = float(np.float32(np.exp(-0.1)))          # SRM ratio
    k2 = float(np.float32(np.exp(1.0) / 200.0))  # a2 pre-scale: u2/20 = sum
    t = np.arange(T, dtype=np.float64)
    srm = (t / 10.0) * np.exp(1.0 - t / 10.0)
    c20 = (np.cumsum(srm) / 20.0).astype(np.float32)
    return q, p, k2, c20


def build():
    import concourse.bass as bass
    import concourse.bacc as bacc
    import concourse.mybir as mybir
    from concourse import tile

    f32 = mybir.dt.float32
    Alu = mybir.AluOpType
    Act = mybir.ActivationFunctionType
    q, p, k2, c20 = _consts()

    nc = bacc.Bacc("TRN2", target_bir_lowering=False, debug=False, num_devices=8)

    w1p_d = nc.dram_tensor("w1p", [128, 4, KC * NO1], f32, kind="ExternalInput")
    fTp_d = nc.dram_tensor("fTp", [128, KT * NB], f32, kind="ExternalInput")
    w2p_d = nc.dram_tensor("w2p", [PMAX, 4, NO2], f32, kind="ExternalInput")
    pc_d = nc.dram_tensor("pconst", [84, B], f32, kind="ExternalInput")
    eye_d = nc.dram_tensor("eye8", [NB, NB], f32, kind="ExternalInput")
    sel_d = nc.dram_tensor("sel32", [NO2, 3, 32], f32, kind="ExternalInput")
    out_d = nc.dram_tensor("out", [84, T], f32, kind="ExternalOutput")

    with tile.TileContext(nc) as tc:
        with (
            tc.tile_pool(name="pers", bufs=1) as pool,
            tc.tile_pool(name="ps1", bufs=1, space="PSUM") as ps1,
            tc.tile_pool(name="ps2", bufs=2, space="PSUM") as ps2,
        ):
            w1sb = pool.tile([128, KT, NO1], f32, tag="w1sb")
            fTsb = pool.tile([128, KT, NB], f32, tag="fTsb")
            w2sb = pool.tile([PMAX, 4, NO2], f32, tag="w2sb")
            pcsb = pool.tile([84, B], f32, tag="pcsb")
            eyesb = pool.tile([NB, NB], f32, tag="eyesb")
            selsb = pool.tile([NO2, 3, 32], f32, tag="selsb")
            a1rsb = pool.tile([NB, NO1], f32, tag="a1rsb")
            A1 = pool.tile([PMAX, 32], f32, tag="A1")
            Up = pool.tile([PMAX, TF, NCOL], f32, tag="Up")
            # mega-tile: spike history S (TF+1 slots of NCOL) followed by the
            # IIR state [P(NCOL) | R(NCOL)].
            SW = (TF + 1) * NCOL
            M = pool.tile([PMAX, SW + 2 * NCOL], f32, tag="M")
            a2tmp = pool.tile([NO2, B, NB], f32, tag="a2tmp")
            a2s = pool.tile([84, T + 1], f32, tag="a2s")
            W1 = pool.tile([84, T + 1], f32, tag="W1")
            W2 = pool.tile([84, T + 1], f32, tag="W2")
            ostage = pool.tile([84, T], f32, tag="ostage")

            # ---- input DMAs: w1 chunks + flat on the gpsimd queue (cheap
            # triggers), small constants on sync ----
            for c in range(4):
                nc.gpsimd.dma_start(
                    w1sb[:, KC * c:KC * (c + 1), :], w1p_d[:, c, :]
                )
            nc.gpsimd.dma_start(fTsb[:], fTp_d[:])
            nc.sync.dma_start(w2sb[:], w2p_d[:])
            nc.sync.dma_start(pcsb[:], pc_d[:])
            nc.sync.dma_start(eyesb[:], eye_d[:])
            nc.sync.dma_start(selsb[:], sel_d[:])

            # ---- state init (rides during DMA) ----
            nc.vector.memset(M[:, 0:NCOL], 0.0)            # S slot 0
            nc.vector.memset(M[:, SW:SW + 2 * NCOL], 0.0)  # P | R
            nc.vector.memset(A1[:], 0.0)
            # layer-2 u column (incl. garbage rows) — NaN insurance
            UW = Up.ap[0][0]
            nc.gpsimd.memset(
                bass.AP(Up.tensor, Up.offset + 32, [[UW, PMAX], [NCOL, TF]]),
                0.0,
            )
            nc.gpsimd.memset(a2s[:, 0:1], 0.0)
            nc.gpsimd.memset(W1[:, 0:1], 0.0)
            nc.gpsimd.memset(W2[:, 0:1], 0.0)

            # ---- fc1: a1row[n, o] = flat @ w1.T, accumulated over k ----
            a1row = ps1.tile([NB, NO1], f32, tag="a1row", name="a1row")
            for k in range(KT):
                nc.tensor.matmul(
                    a1row[:], fTsb[:, k, :], w1sb[:, k, :],
                    start=(k == 0), stop=(k == KT - 1),
                )
            nc.scalar.activation(a1rsb[:], a1row[:], Act.Copy)
            # transpose to scan layout: A1[o_chunk, (c, n)]
            for c in range(4):
                a1tp = ps1.tile([PMAX, NB], f32, tag=f"a1tp{c % 2}", name="a1tp")
                nc.tensor.transpose(
                    a1tp[0:MC[c], :],
                    a1rsb[0:NB, OFF[c]:OFF[c] + MC[c]],
                    eyesb[:],
                )
                nc.scalar.activation(
                    A1[0:MC[c], c * NB:(c + 1) * NB], a1tp[0:MC[c], :], Act.Copy
                )

            # ---- block/scalar-stream schedules ----
            blocks = []
            for b in range((T + B - 1) // B):
                blocks.append((b * B, min((b + 1) * B, T)))
            blk_at = {tb1 - 1: (bi, tb0, tb1)
                      for bi, (tb0, tb1) in enumerate(blocks)}
            wscan_at = {}
            for bi, (tb0, tb1) in enumerate(blocks):
                wscan_at.setdefault(min(tb1 - 1 + DELAY, T - 1), []).append(bi)
            evac_at = {}
            for bi, (tb0, tb1) in enumerate(blocks):
                evac_at.setdefault(min(tb1 - 1 + EVAC1_DELAY, T - 1), []).append(
                    (bi, 1))
                evac_at.setdefault(min(tb1 - 1 + EVAC2_DELAY, T - 1), []).append(
                    (bi, 2))

            a2ps_t = {}
            a2r_t = {}

            def emit_evac(bi, which):
                tb0, tb1 = blocks[bi]
                blk = tb1 - tb0
                if which == 1:
                    nc.scalar.activation(
                        a2tmp[:, 0:blk, :], a2ps_t[bi][:, 0:blk, :],
                        Act.Copy, scale=k2,
                    )
                else:
                    nc.scalar.activation(
                        a2s[:, tb0 + 1:tb1 + 1], a2r_t[bi][0:84, 0:blk],
                        Act.Copy,
                    )

            def emit_restack(bi):
                tb0, tb1 = blocks[bi]
                blk = tb1 - tb0
                # re-stack to gapped rows (r = 32g+10j+o2): shared shifted
                # identities, PSUM partition bases 0/32/64
                a2r = ps2.tile([96, B], f32, tag="a2r", name="a2r")
                a2r_t[bi] = a2r
                for g in range(3):
                    nj = min(3, NB - 3 * g)
                    for j in range(nj):
                        nc.tensor.matmul(
                            a2r[32 * g:32 * g + 32, 0:blk],
                            selsb[:, j, :],
                            a2tmp[:, 0:blk, 3 * g + j],
                            start=(j == 0), stop=(j == nj - 1),
                        )

            def emit_wscan(bi):
                tb0, tb1 = blocks[bi]
                blk = tb1 - tb0
                # W1[t] = (a2s[t-1] + W1[t-1]) * p   (hardware scan)
                nc.vector.tensor_tensor_scan(
                    W1[:, tb0 + 1:tb1 + 1], a2s[:, tb0:tb1], pcsb[:, 0:blk],
                    W1[:, tb0:tb0 + 1], Alu.add, Alu.mult,
                )
                nc.vector.tensor_tensor_scan(
                    W2[:, tb0 + 1:tb1 + 1], W1[:, tb0:tb1], pcsb[:, 0:blk],
                    W2[:, tb0:tb0 + 1], Alu.add, Alu.mult,
                )
                # u2/20 = W1 + W2 -> layer-2 column of Up, lagged by LAG
                nc.gpsimd.tensor_tensor(
                    Up[0:84, tb0 + LAG:tb1 + LAG, 32],
                    W1[:, tb0 + 1:tb1 + 1], W2[:, tb0 + 1:tb1 + 1], Alu.add,
                )

            # ---- fused scan: layer-1 at step tau, layer-2 at tau-LAG ----
            MW = M.ap[0][0]          # mega-tile row stride (elements)
            MOFF = M.offset

            def m_ap(off, dims, parts=PMAX):
                return bass.AP(M.tensor, MOFF + off, [[MW, parts]] + dims)

            def emit_uprow(s):
                nc.scalar.activation(
                    Up[:, s, 0:32], A1[:], Act.Copy, scale=float(c20[s])
                )

            AHEAD = 6  # scalar threshold-row emission lead over the DVE scan
            for s in range(AHEAD):
                emit_uprow(s)

            for tau in range(TF):
                if tau < T:
                    pp = m_ap(SW, [[1, NCOL]])
                    rr = m_ap(SW + NCOL, [[1, NCOL]])
                    w0, off = NCOL, 0
                else:
                    # tail: layer-1 finished, only column 32 is live
                    pp = m_ap(SW + 32, [[1, 1]])
                    rr = m_ap(SW + NCOL + 32, [[1, 1]])
                    w0, off = 1, 32
                # P = q*P + s_{tau-1}
                nc.vector.scalar_tensor_tensor(
                    pp, pp, q, m_ap(tau * NCOL + off, [[1, w0]]),
                    Alu.mult, Alu.add,
                )
                # R = q*R + P
                nc.vector.scalar_tensor_tensor(
                    rr, rr, q, pp, Alu.mult, Alu.add,
                )
                # s_{tau} = (R + 0.5) <= u/20
                nc.vector.scalar_tensor_tensor(
                    m_ap((tau + 1) * NCOL + off, [[1, w0]]),
                    rr, 0.5, Up[:, tau, off:off + w0], Alu.add, Alu.is_le,
                )

                if tau < T:
                    # PE fc2 block launch (waits on spike history via sems)
                    if tau in blk_at:
                        bi, tb0, tb1 = blk_at[tau]
                        blk = tb1 - tb0
                        a2ps = ps2.tile([NO2, B, NB], f32, tag="a2ps",
                                        name="a2ps")
                        a2ps_t[bi] = a2ps
                        for c in range(4):
                            nc.tensor.matmul(
                                a2ps[:, 0:blk, :],
                                w2sb[:, c, :],
                                m_ap((tb0 + 1) * NCOL + c * NB,
                                     [[NCOL, blk], [1, NB]]),
                                start=(c == 0), stop=(c == 3),
                            )
                    # scalar stream: threshold row + due evacuations (each
                    # evac is followed immediately by its PE/DVE consumers so
                    # cross-queue program order matches data order)
                    if tau + AHEAD < T:
                        emit_uprow(tau + AHEAD)
                    for bi, which in evac_at.get(tau, []):
                        emit_evac(bi, which)
                        if which == 1:
                            emit_restack(bi)
                    # DVE stream: delayed PSP scans
                    for bi in wscan_at.get(tau, []):
                        emit_wscan(bi)

            # ---- output: layer-2 spikes, fused steps LAG..LAG+T ----
            nc.vector.tensor_copy(
                ostage[:],
                m_ap((LAG + 1) * NCOL + 32, [[NCOL, T]], parts=84),
            )
            nc.sync.dma_start(out_d[:], ostage[:])

    nc.compile()
    return nc


def _host_inputs(input, w1, w2):
    f32 = np.float32
    q, p, k2, c20 = _consts()
    flat = np.ascontiguousarray(input.reshape(64, -1).astype(f32))
    # fTp[p, k, n] = flat[n, k*128+p]
    fTp = np.ascontiguousarray(
        flat.T.reshape(KT, 128, 64).transpose(1, 0, 2))  # (128, KT, 64)
    # w1p[p, c, kc*410+o] = w1[o, (c*KC+kc)*128+p]
    w1T = w1.astype(f32).T.reshape(4, KC, 128, NO1)       # (c, kc, p, o)
    w1p = np.ascontiguousarray(
        w1T.transpose(2, 0, 1, 3).reshape(128, 4, KC * NO1))
    w2p = np.zeros((PMAX, 4, NO2), f32)
    for c in range(4):
        w2p[0:MC[c], c, :] = w2.astype(f32)[:, OFF[c]:OFF[c] + MC[c]].T
    pconst = np.full((84, B), p, f32)
    eye8 = np.eye(NB, dtype=f32)
    sel32 = np.zeros((NO2, 3, 32), f32)
    for j in range(3):
        for o2 in range(NO2):
            sel32[o2, j, 10 * j + o2] = 1.0
    return fTp, w1p, w2p, pconst, eye8, sel32


def kernel(input, w1, w2):
    from concourse.bass_utils import run_bass_kernel_spmd

    if "nc" not in _CACHE:
        _CACHE["nc"] = build()
    nc = _CACHE["nc"]

    fTp, w1p, w2p, pconst, eye8, sel32 = _host_inputs(input, w1, w2)
    in_maps = []
    for core in range(8):
        fTc = fTp[:, :, core * NB:(core + 1) * NB].reshape(128, KT * NB)
        in_maps.append({
            "fTp": np.ascontiguousarray(fTc),
            "w1p": w1p,
            "w2p": w2p,
            "pconst": pconst,
            "eye8": eye8,
            "sel32": sel32,
        })
    res = run_bass_kernel_spmd(nc, in_maps, core_ids=list(range(8)))
    # row r = 32*g + 10*j + o2 holds batch n = 3*g + j
    rows = np.array([32 * (n // 3) + 10 * (n % 3) + np.arange(NO2)
                     for n in range(NB)])          # (8, 10)
    full = np.zeros((64, NO2, T), np.float32)
    for core in range(8):
        o = res.results[core]["out"]               # (84, T)
        full[core * NB:(core + 1) * NB] = o[rows]
    return full


# revision 14
# speedup vs baseline: 1.1252x; 1.0516x over previous
"""Trainium2 Bass kernel for the SLAYER-style 2-layer spiking MLP.

Reference computation (per batch element n):
    flat   = input.reshape(64, 3072)
    a1     = flat @ w1.T                      (constant over time)
    u1[t]  = a1 * c[t]          where c = cumsum(srm kernel)  (PSP of a
             time-constant input is just a ramp scale)
    s1     = spike_scan(u1)     sequential threshold w/ refractory feedback
    a2[t]  = w2 @ s1[:, t]
    u2     = psp(a2)            (true temporal conv, srm kernel)
    out    = spike_scan(u2)

Refractory feedback is an exact order-2 IIR (kernel rk[d] = -20 d e^{1-d}):
    P[t] = q*P[t-1] + s[t-1];  R[t] = q*R[t-1] + P[t]   (q = e^-1)
    spike:  (R + 0.5) <= u/20
Each fused scan step is 3 scalar_tensor_tensor DVE ops over a [104, 33]
tile holding both layers (layer 2 rides along lagged LAG steps).

Schedule highlights vs the naive version:
  * w1 / flat are host-packed so each DMA moves long contiguous
    per-partition lines (128 descriptors instead of 3072).
  * The u1/20 threshold rows are produced per-step on the otherwise-idle
    Scalar engine (activation Copy, scale=c20[t]) instead of one huge
    broadcast-AP DVE op that serialized the whole prologue.
  * PSUM evacuations ride the scalar stream at delayed slots so they
    never block threshold-row production.
  * fc2 re-stack uses a shared eye(10) stationary with per-n PSUM
    partition-offset writes (1 LDWEIGHTS instead of 8 full selector
    loads per block).

Sharding: data-parallel over batch, 8 elements per core, weights replicated.
"""

import numpy as np

NB = 8            # batch elements per core
T = 100           # timesteps
B = 16            # pipeline block size
LAG = 32          # layer-2 ride-along lag (>= block + pipeline latency)
TF = T + LAG      # fused scan steps
NCOL = 33         # 32 layer-1 columns (4 chunks x 8 batch) + 1 layer-2 column
PMAX = 104        # padded partition count per o-chunk
MC = [103, 103, 102, 102]      # o-chunk sizes (sum = 410)
OFF = [0, 103, 206, 308]
KT = 24           # 3072 / 128 k-tiles
KCS = [3, 7, 7, 7]  # k-tiles per w1 DMA chunk (small first chunk)
NO1 = 410
NO2 = 10
DELAY = 16        # DVE-stream slots after block end before W1/W2 scans
EVAC1_DELAY = 8   # stream slots after block end before PSUM evac 1
EVAC2_DELAY = 15  # (selector MMs are emitted right after evac1)
UCHUNK = 12       # threshold-surface timesteps per gpsimd instruction
OSPLIT = 64       # output columns staged/DMA'd early, overlapping the tail

_CACHE = {}


def _consts():
    q = float(np.float32(np.exp(-1.0)))          # refractory ratio
    p = float(np.float32(np.exp(-0.1)))          # SRM ratio
    k2 = float(np.float32(np.exp(1.0) / 200.0))  # a2 pre-scale: u2/20 = sum
    t = np.arange(T, dtype=np.float64)
    srm = (t / 10.0) * np.exp(1.0 - t / 10.0)
    c20 = (np.cumsum(srm) / 20.0).astype(np.float32)
    return q, p, k2, c20


def build():
    import concourse.bass as bass
    import concourse.bacc as bacc
    import concourse.mybir as mybir
    from concourse import tile

    f32 = mybir.dt.float32
    Alu = mybir.AluOpType
    Act = mybir.ActivationFunctionType
    q, p, k2, c20 = _consts()

    nc = bacc.Bacc("TRN2", target_bir_lowering=False, debug=False, num_devices=8)

    w1p_d = nc.dram_tensor("w1p", [128, KT * NO1], f32, kind="ExternalInput")
    fTp_d = nc.dram_tensor("fTp", [128, KT * NB], f32, kind="ExternalInput")
    w2p_d = nc.dram_tensor("w2p", [PMAX, 4, NO2], f32, kind="ExternalInput")
    pc_d = nc.dram_tensor("pconst", [84, B], f32, kind="ExternalInput")
    eye_d = nc.dram_tensor("eye8", [NB, NB], f32, kind="ExternalInput")
    sel_d = nc.dram_tensor("sel32", [NO2, 3, 32], f32, kind="ExternalInput")
    c20_d = nc.dram_tensor("c20rep", [PMAX, T], f32, kind="ExternalInput")
    out_d = nc.dram_tensor("out", [84, T], f32, kind="ExternalOutput")

    with tile.TileContext(nc) as tc:
        with (
            tc.tile_pool(name="pers", bufs=1) as pool,
            tc.tile_pool(name="ps1", bufs=1, space="PSUM") as ps1,
            tc.tile_pool(name="ps2", bufs=2, space="PSUM") as ps2,
        ):
            w1sb = pool.tile([128, KT, NO1], f32, tag="w1sb")
            fTsb = pool.tile([128, KT, NB], f32, tag="fTsb")
            w2sb = pool.tile([PMAX, 4, NO2], f32, tag="w2sb")
            pcsb = pool.tile([84, B], f32, tag="pcsb")
            eyesb = pool.tile([NB, NB], f32, tag="eyesb")
            selsb = pool.tile([NO2, 3, 32], f32, tag="selsb")
            a1rsb = pool.tile([NB, NO1], f32, tag="a1rsb")
            A1 = pool.tile([PMAX, 32], f32, tag="A1")
            Up = pool.tile([PMAX, NCOL, TF], f32, tag="Up")
            c20sb = pool.tile([PMAX, T], f32, tag="c20sb")
            # mega-tile: spike history S (TF+1 slots of NCOL) followed by the
            # IIR state [P(NCOL) | R(NCOL)].
            SW = (TF + 1) * NCOL
            M = pool.tile([PMAX, SW + 2 * NCOL], f32, tag="M")
            a2tmp = pool.tile([NO2, B, NB], f32, tag="a2tmp")
            a2s = pool.tile([84, T + 1], f32, tag="a2s")
            W1 = pool.tile([84, T + 1], f32, tag="W1")
            W2 = pool.tile([84, T + 1], f32, tag="W2")
            ostage = pool.tile([84, T], f32, tag="ostage")

            # ---- input DMAs: flat first (gates fc1's stationary), then w1
            # chunks (small chunk 0 so fc1 starts early); constants on sync ----
            nc.gpsimd.dma_start(fTsb[:], fTp_d[:])
            ko = 0
            for kc in KCS:
                nc.gpsimd.dma_start(
                    w1sb[:, ko:ko + kc, :],
                    w1p_d[:, ko * NO1:(ko + kc) * NO1],
                )
                ko += kc
            nc.sync.dma_start(w2sb[:], w2p_d[:])
            nc.sync.dma_start(pcsb[:], pc_d[:])
            nc.sync.dma_start(eyesb[:], eye_d[:])
            nc.sync.dma_start(selsb[:], sel_d[:])
            nc.sync.dma_start(c20sb[:], c20_d[:])

            # ---- state init (rides during DMA) ----
            nc.vector.memset(M[:, 0:NCOL], 0.0)            # S slot 0
            nc.vector.memset(M[:, SW:SW + 2 * NCOL], 0.0)  # P | R
            nc.vector.memset(A1[:], 0.0)
            # layer-2 u column (incl. garbage rows) — NaN insurance
            UW = Up.ap[0][0]
            nc.gpsimd.memset(Up[:, 32, :], 0.0)
            nc.gpsimd.memset(a2s[:, 0:1], 0.0)
            nc.gpsimd.memset(W1[:, 0:1], 0.0)
            nc.gpsimd.memset(W2[:, 0:1], 0.0)

            # ---- fc1: a1row[n, o] = flat @ w1.T, accumulated over k ----
            a1row = ps1.tile([NB, NO1], f32, tag="a1row", name="a1row")
            for k in range(KT):
                nc.tensor.matmul(
                    a1row[:], fTsb[:, k, :], w1sb[:, k, :],
                    start=(k == 0), stop=(k == KT - 1),
                )
            nc.scalar.activation(a1rsb[:], a1row[:], Act.Copy)
            # transpose to scan layout: A1[o_chunk, (c, n)]
            for c in range(4):
                a1tp = ps1.tile([PMAX, NB], f32, tag=f"a1tp{c % 2}", name="a1tp")
                nc.tensor.transpose(
                    a1tp[0:MC[c], :],
                    a1rsb[0:NB, OFF[c]:OFF[c] + MC[c]],
                    eyesb[:],
                )
                nc.scalar.activation(
                    A1[0:MC[c], c * NB:(c + 1) * NB], a1tp[0:MC[c], :], Act.Copy
                )

            # ---- block/scalar-stream schedules ----
            blocks = []
            for b in range((T + B - 1) // B):
                blocks.append((b * B, min((b + 1) * B, T)))
            blk_at = {tb1 - 1: (bi, tb0, tb1)
                      for bi, (tb0, tb1) in enumerate(blocks)}
            wscan_at = {}
            for bi, (tb0, tb1) in enumerate(blocks):
                wscan_at.setdefault(min(tb1 - 1 + DELAY, T - 1), []).append(bi)
            evac_at = {}
            for bi, (tb0, tb1) in enumerate(blocks):
                evac_at.setdefault(min(tb1 - 1 + EVAC1_DELAY, T - 1), []).append(
                    (bi, 1))
                evac_at.setdefault(min(tb1 - 1 + EVAC2_DELAY, T - 1), []).append(
                    (bi, 2))

            a2ps_t = {}
            a2r_t = {}

            def emit_evac(bi, which):
                tb0, tb1 = blocks[bi]
                blk = tb1 - tb0
                if which == 1:
                    nc.scalar.activation(
                        a2tmp[:, 0:blk, :], a2ps_t[bi][:, 0:blk, :],
                        Act.Copy, scale=k2,
                    )
                else:
                    nc.scalar.activation(
                        a2s[:, tb0 + 1:tb1 + 1], a2r_t[bi][0:84, 0:blk],
                        Act.Copy,
                    )

            def emit_restack(bi):
                tb0, tb1 = blocks[bi]
                blk = tb1 - tb0
                # re-stack to gapped rows (r = 32g+10j+o2): shared shifted
                # identities, PSUM partition bases 0/32/64
                a2r = ps2.tile([96, B], f32, tag="a2r", name="a2r")
                a2r_t[bi] = a2r
                for g in range(3):
                    nj = min(3, NB - 3 * g)
                    for j in range(nj):
                        nc.tensor.matmul(
                            a2r[32 * g:32 * g + 32, 0:blk],
                            selsb[:, j, :],
                            a2tmp[:, 0:blk, 3 * g + j],
                            start=(j == 0), stop=(j == nj - 1),
                        )

            def emit_wscan(bi):
                tb0, tb1 = blocks[bi]
                blk = tb1 - tb0
                # W1[t] = (a2s[t-1] + W1[t-1]) * p   (hardware scan)
                nc.vector.tensor_tensor_scan(
                    W1[:, tb0 + 1:tb1 + 1], a2s[:, tb0:tb1], pcsb[:, 0:blk],
                    W1[:, tb0:tb0 + 1], Alu.add, Alu.mult,
                )
                nc.vector.tensor_tensor_scan(
                    W2[:, tb0 + 1:tb1 + 1], W1[:, tb0:tb1], pcsb[:, 0:blk],
                    W2[:, tb0:tb0 + 1], Alu.add, Alu.mult,
                )
                # u2/20 = W1 + W2 -> layer-2 column of Up, lagged by LAG
                nc.gpsimd.tensor_tensor(
                    Up[0:84, 32, tb0 + LAG:tb1 + LAG],
                    W1[:, tb0 + 1:tb1 + 1], W2[:, tb0 + 1:tb1 + 1], Alu.add,
                )

            # ---- fused scan: layer-1 at step tau, layer-2 at tau-LAG ----
            MW = M.ap[0][0]          # mega-tile row stride (elements)
            MOFF = M.offset

            def m_ap(off, dims, parts=PMAX):
                return bass.AP(M.tensor, MOFF + off, [[MW, parts]] + dims)

            def emit_uchunk(g0):
                if g0 >= T:
                    return
                g1 = min(g0 + UCHUNK, T)
                w = g1 - g0
                # Up[p, j, t] = A1[p, j] * c20[t] — broadcast outer product
                nc.gpsimd.tensor_tensor(
                    Up[:, 0:32, g0:g1],
                    A1[:].unsqueeze(2).broadcast_to([PMAX, 32, w]),
                    c20sb[:, g0:g1].unsqueeze(1).broadcast_to([PMAX, 32, w]),
                    Alu.mult,
                )

            emit_uchunk(0)
            emit_uchunk(UCHUNK)

            for tau in range(TF):
                if tau < T:
                    pp = m_ap(SW, [[1, NCOL]])
                    rr = m_ap(SW + NCOL, [[1, NCOL]])
                    w0, off = NCOL, 0
                else:
                    # tail: layer-1 finished, only column 32 is live
                    pp = m_ap(SW + 32, [[1, 1]])
                    rr = m_ap(SW + NCOL + 32, [[1, 1]])
                    w0, off = 1, 32
                # P = q*P + s_{tau-1}
                nc.vector.scalar_tensor_tensor(
                    pp, pp, q, m_ap(tau * NCOL + off, [[1, w0]]),
                    Alu.mult, Alu.add,
                )
                # R = q*R + P
                nc.vector.scalar_tensor_tensor(
                    rr, rr, q, pp, Alu.mult, Alu.add,
                )
                # s_{tau} = (R + 0.5) <= u/20
                nc.vector.scalar_tensor_tensor(
                    m_ap((tau + 1) * NCOL + off, [[1, w0]]),
                    rr, 0.5,
                    bass.AP(Up.tensor, Up.offset + off * TF + tau,
                            [[UW, PMAX], [TF, w0]]),
                    Alu.add, Alu.is_le,
                )

                if tau == OSPLIT - 1 + LAG and tau >= T:
                    nc.scalar.activation(
                        ostage[:, 0:OSPLIT],
                        m_ap((LAG + 1) * NCOL + 32, [[NCOL, OSPLIT]],
                             parts=84),
                        Act.Copy,
                    )
                    nc.sync.dma_start(out_d[:, 0:OSPLIT], ostage[:, 0:OSPLIT])
                if tau < T:
                    # PE fc2 block launch (waits on spike history via sems)
                    if tau in blk_at:
                        bi, tb0, tb1 = blk_at[tau]
                        blk = tb1 - tb0
                        a2ps = ps2.tile([NO2, B, NB], f32, tag="a2ps",
                                        name="a2ps")
                        a2ps_t[bi] = a2ps
                        for c in range(4):
                            nc.tensor.matmul(
                                a2ps[:, 0:blk, :],
                                w2sb[:, c, :],
                                m_ap((tb0 + 1) * NCOL + c * NB,
                                     [[NCOL, blk], [1, NB]]),
                                start=(c == 0), stop=(c == 3),
                            )
                    # threshold chunks stay 2 ahead of the scan
                    if tau % UCHUNK == 0 and tau + 2 * UCHUNK < T + UCHUNK:
                        emit_uchunk(tau + 2 * UCHUNK)
                    if tau == OSPLIT - 1 + LAG:
                        # early output half overlaps the remaining tail
                        nc.scalar.activation(
                            ostage[:, 0:OSPLIT],
                            m_ap((LAG + 1) * NCOL + 32, [[NCOL, OSPLIT]],
                                 parts=84),
                            Act.Copy,
                        )
                        nc.sync.dma_start(out_d[:, 0:OSPLIT],
                                          ostage[:, 0:OSPLIT])
                    for bi, which in evac_at.get(tau, []):
                        emit_evac(bi, which)
                        if which == 1:
                            emit_restack(bi)
                    # DVE stream: delayed PSP scans
                    for bi in wscan_at.get(tau, []):
                        emit_wscan(bi)

            # ---- output: remaining layer-2 spikes ----
            nc.scalar.activation(
                ostage[:, OSPLIT:T],
                m_ap((OSPLIT + LAG + 1) * NCOL + 32, [[NCOL, T - OSPLIT]],
                     parts=84),
                Act.Copy,
            )
            nc.sync.dma_start(out_d[:, OSPLIT:T], ostage[:, OSPLIT:T])

    nc.compile()
    return nc


def _host_inputs(input, w1, w2):
    f32 = np.float32
    q, p, k2, c20 = _consts()
    flat = np.ascontiguousarray(input.reshape(64, -1).astype(f32))
    # fTp[p, k, n] = flat[n, k*128+p]
    fTp = np.ascontiguousarray(
        flat.T.reshape(KT, 128, 64).transpose(1, 0, 2))  # (128, KT, 64)
    # w1p[p, k*410+o] = w1[o, k*128+p]
    w1T = w1.astype(f32).T.reshape(KT, 128, NO1)          # (k, p, o)
    w1p = np.ascontiguousarray(
        w1T.transpose(1, 0, 2).reshape(128, KT * NO1))
    w2p = np.zeros((PMAX, 4, NO2), f32)
    for c in range(4):
        w2p[0:MC[c], c, :] = w2.astype(f32)[:, OFF[c]:OFF[c] + MC[c]].T
    pconst = np.full((84, B), p, f32)
    eye8 = np.eye(NB, dtype=f32)
    sel32 = np.zeros((NO2, 3, 32), f32)
    for j in range(3):
        for o2 in range(NO2):
            sel32[o2, j, 10 * j + o2] = 1.0
    c20rep = np.broadcast_to(c20, (PMAX, T)).copy()
    return fTp, w1p, w2p, pconst, eye8, sel32, c20rep


def kernel(input, w1, w2):
    from concourse.bass_utils import run_bass_kernel_spmd

    if "nc" not in _CACHE:
        _CACHE["nc"] = build()
    nc = _CACHE["nc"]

    fTp, w1p, w2p, pconst, eye8, sel32, c20rep = _host_inputs(input, w1, w2)
    in_maps = []
    for core in range(8):
        fTc = fTp[:, :, core * NB:(core + 1) * NB].reshape(128, KT * NB)
        in_maps.append({
            "fTp": np.ascontiguousarray(fTc),
            "w1p": w1p,
            "w2p": w2p,
            "pconst": pconst,
            "eye8": eye8,
            "sel32": sel32,
            "c20rep": c20rep,
        })
    res = run_bass_kernel_spmd(nc, in_maps, core_ids=list(range(8)))
    # row r = 32*g + 10*j + o2 holds batch n = 3*g + j
    rows = np.array([32 * (n // 3) + 10 * (n % 3) + np.arange(NO2)
                     for n in range(NB)])          # (8, 10)
    full = np.zeros((64, NO2, T), np.float32)
    for core in range(8):
        o = res.results[core]["out"]               # (84, T)
        full[core * NB:(core + 1) * NB] = o[rows]
    return full


# revision 16
# speedup vs baseline: 1.1300x; 1.0042x over previous
"""Trainium2 Bass kernel for the SLAYER-style 2-layer spiking MLP.

Reference computation (per batch element n):
    flat   = input.reshape(64, 3072)
    a1     = flat @ w1.T                      (constant over time)
    u1[t]  = a1 * c[t]          where c = cumsum(srm kernel)  (PSP of a
             time-constant input is just a ramp scale)
    s1     = spike_scan(u1)     sequential threshold w/ refractory feedback
    a2[t]  = w2 @ s1[:, t]
    u2     = psp(a2)            (true temporal conv, srm kernel)
    out    = spike_scan(u2)

Refractory feedback is an exact order-2 IIR (kernel rk[d] = -20 d e^{1-d}):
    P[t] = q*P[t-1] + s[t-1];  R[t] = q*R[t-1] + P[t]   (q = e^-1)
    spike:  (R + 0.5) <= u/20
Each fused scan step is 3 scalar_tensor_tensor DVE ops over a [104, 33]
tile holding both layers (layer 2 rides along lagged LAG steps).

Schedule highlights vs the naive version:
  * w1 / flat are host-packed so each DMA moves long contiguous
    per-partition lines (128 descriptors instead of 3072).
  * The u1/20 threshold rows are produced per-step on the otherwise-idle
    Scalar engine (activation Copy, scale=c20[t]) instead of one huge
    broadcast-AP DVE op that serialized the whole prologue.
  * PSUM evacuations ride the scalar stream at delayed slots so they
    never block threshold-row production.
  * fc2 re-stack uses a shared eye(10) stationary with per-n PSUM
    partition-offset writes (1 LDWEIGHTS instead of 8 full selector
    loads per block).

Sharding: data-parallel over batch, 8 elements per core, weights replicated.
"""

import numpy as np

NB = 8            # batch elements per core
T = 100           # timesteps
B = 16            # pipeline block size
LAG = 32          # layer-2 ride-along lag (>= block + pipeline latency)
TF = T + LAG      # fused scan steps
NCOL = 33         # 32 layer-1 columns (4 chunks x 8 batch) + 1 layer-2 column
PMAX = 104        # padded partition count per o-chunk
MC = [103, 103, 102, 102]      # o-chunk sizes (sum = 410)
OFF = [0, 103, 206, 308]
KT = 24           # 3072 / 128 k-tiles
KCS = [1, 7, 8, 8]  # k-tiles per w1 DMA chunk (small first chunk)
NO1 = 410
NO2 = 10
DELAY = 16        # DVE-stream slots after block end before W1/W2 scans
EVAC1_DELAY = 8   # stream slots after block end before PSUM evac 1
EVAC2_DELAY = 15  # (selector MMs are emitted right after evac1)
UCHUNK = 12       # threshold-surface timesteps per gpsimd instruction
OSPLIT = 88       # output columns staged/DMA'd early, overlapping the tail

_CACHE = {}


def _consts():
    q = float(np.float32(np.exp(-1.0)))          # refractory ratio
    p = float(np.float32(np.exp(-0.1)))          # SRM ratio
    k2 = float(np.float32(np.exp(1.0) / 200.0))  # a2 pre-scale: u2/20 = sum
    t = np.arange(T, dtype=np.float64)
    srm = (t / 10.0) * np.exp(1.0 - t / 10.0)
    c20 = (np.cumsum(srm) / 20.0).astype(np.float32)
    return q, p, k2, c20


def build():
    import concourse.bass as bass
    import concourse.bacc as bacc
    import concourse.mybir as mybir
    from concourse import tile

    f32 = mybir.dt.float32
    Alu = mybir.AluOpType
    Act = mybir.ActivationFunctionType
    q, p, k2, c20 = _consts()

    nc = bacc.Bacc("TRN2", target_bir_lowering=False, debug=False, num_devices=8)

    w1p_d = nc.dram_tensor("w1p", [128, KT * NO1], f32, kind="ExternalInput")
    fTp_d = nc.dram_tensor("fTp", [128, KT * NB], f32, kind="ExternalInput")
    w2p_d = nc.dram_tensor("w2p", [PMAX, 4, NO2], f32, kind="ExternalInput")
    pc_d = nc.dram_tensor("pconst", [84, B], f32, kind="ExternalInput")
    eye_d = nc.dram_tensor("eye8", [NB, NB], f32, kind="ExternalInput")
    sel_d = nc.dram_tensor("sel32", [NO2, 3, 32], f32, kind="ExternalInput")
    c20_d = nc.dram_tensor("c20rep", [PMAX, T], f32, kind="ExternalInput")
    out_d = nc.dram_tensor("out", [84, T], f32, kind="ExternalOutput")

    with tile.TileContext(nc) as tc:
        with (
            tc.tile_pool(name="pers", bufs=1) as pool,
            tc.tile_pool(name="ps1", bufs=1, space="PSUM") as ps1,
            tc.tile_pool(name="ps2", bufs=2, space="PSUM") as ps2,
        ):
            w1sb = pool.tile([128, KT, NO1], f32, tag="w1sb")
            fTsb = pool.tile([128, KT, NB], f32, tag="fTsb")
            w2sb = pool.tile([PMAX, 4, NO2], f32, tag="w2sb")
            pcsb = pool.tile([84, B], f32, tag="pcsb")
            eyesb = pool.tile([NB, NB], f32, tag="eyesb")
            selsb = pool.tile([NO2, 3, 32], f32, tag="selsb")
            a1rsb = pool.tile([NB, NO1], f32, tag="a1rsb")
            A1 = pool.tile([PMAX, 32], f32, tag="A1")
            Up = pool.tile([PMAX, NCOL, TF], f32, tag="Up")
            c20sb = pool.tile([PMAX, T], f32, tag="c20sb")
            # mega-tile: spike history S (TF+1 slots of NCOL) followed by the
            # IIR state [P(NCOL) | R(NCOL)].
            SW = (TF + 1) * NCOL
            M = pool.tile([PMAX, SW + 2 * NCOL], f32, tag="M")
            a2tmp = pool.tile([NO2, B, NB], f32, tag="a2tmp")
            a2s = pool.tile([84, T + 1], f32, tag="a2s")
            W1 = pool.tile([84, T + 1], f32, tag="W1")
            W2 = pool.tile([84, T + 1], f32, tag="W2")
            ostage = pool.tile([84, T], f32, tag="ostage")

            # ---- input DMAs: flat first (gates fc1's stationary), then w1
            # chunks (small chunk 0 so fc1 starts early); constants on sync ----
            nc.gpsimd.dma_start(fTsb[:], fTp_d[:])
            ko = 0
            for kc in KCS:
                nc.gpsimd.dma_start(
                    w1sb[:, ko:ko + kc, :],
                    w1p_d[:, ko * NO1:(ko + kc) * NO1],
                )
                ko += kc
            nc.sync.dma_start(w2sb[:], w2p_d[:])
            nc.sync.dma_start(pcsb[:], pc_d[:])
            nc.sync.dma_start(eyesb[:], eye_d[:])
            nc.sync.dma_start(selsb[:], sel_d[:])
            nc.sync.dma_start(c20sb[:], c20_d[:])

            # ---- state init (rides during DMA) ----
            nc.vector.memset(M[:, 0:NCOL], 0.0)            # S slot 0
            nc.vector.memset(M[:, SW:SW + 2 * NCOL], 0.0)  # P | R
            nc.vector.memset(A1[:], 0.0)
            # layer-2 u column (incl. garbage rows) — NaN insurance
            UW = Up.ap[0][0]
            nc.gpsimd.memset(Up[:, 32, :], 0.0)
            nc.gpsimd.memset(a2s[:, 0:1], 0.0)
            nc.gpsimd.memset(W1[:, 0:1], 0.0)
            nc.gpsimd.memset(W2[:, 0:1], 0.0)

            # ---- PE warmup: ramp the tensor-engine clock on w2sb while the
            # w1 DMA is still in flight (results are discarded) ----
            wrm = ps1.tile([NO2, 40], f32, tag="wrm", name="wrm")
            for _ in range(5):
                nc.tensor.matmul(wrm[:], w2sb[:, 0, :], w2sb[:],
                                 start=True, stop=True)

            # ---- fc1: a1row[n, o] = flat @ w1.T, accumulated over k ----
            a1row = ps1.tile([NB, NO1], f32, tag="a1row", name="a1row")
            for k in range(KT):
                nc.tensor.matmul(
                    a1row[:], fTsb[:, k, :], w1sb[:, k, :],
                    start=(k == 0), stop=(k == KT - 1),
                )
            nc.vector.tensor_copy(a1rsb[:], a1row[:])
            # transpose to scan layout: A1[o_chunk, (c, n)]
            for c in range(4):
                a1tp = ps1.tile([PMAX, NB], f32, tag=f"a1tp{c % 2}", name="a1tp")
                nc.tensor.transpose(
                    a1tp[0:MC[c], :],
                    a1rsb[0:NB, OFF[c]:OFF[c] + MC[c]],
                    eyesb[:],
                )
                nc.vector.tensor_copy(
                    A1[0:MC[c], c * NB:(c + 1) * NB], a1tp[0:MC[c], :]
                )

            # ---- block/scalar-stream schedules ----
            blocks = []
            for b in range((T + B - 1) // B):
                blocks.append((b * B, min((b + 1) * B, T)))
            blk_at = {tb1 - 1: (bi, tb0, tb1)
                      for bi, (tb0, tb1) in enumerate(blocks)}
            wscan_at = {}
            for bi, (tb0, tb1) in enumerate(blocks):
                wscan_at.setdefault(min(tb1 - 1 + DELAY, T - 1), []).append(bi)
            evac_at = {}
            for bi, (tb0, tb1) in enumerate(blocks):
                evac_at.setdefault(min(tb1 - 1 + EVAC1_DELAY, T - 1), []).append(
                    (bi, 1))
                evac_at.setdefault(min(tb1 - 1 + EVAC2_DELAY, T - 1), []).append(
                    (bi, 2))

            a2ps_t = {}
            a2r_t = {}

            def emit_evac(bi, which):
                tb0, tb1 = blocks[bi]
                blk = tb1 - tb0
                if which == 1:
                    nc.scalar.activation(
                        a2tmp[:, 0:blk, :], a2ps_t[bi][:, 0:blk, :],
                        Act.Copy, scale=k2,
                    )
                else:
                    nc.scalar.activation(
                        a2s[:, tb0 + 1:tb1 + 1], a2r_t[bi][0:84, 0:blk],
                        Act.Copy,
                    )

            def emit_restack(bi):
                tb0, tb1 = blocks[bi]
                blk = tb1 - tb0
                # re-stack to gapped rows (r = 32g+10j+o2): shared shifted
                # identities, PSUM partition bases 0/32/64
                a2r = ps2.tile([96, B], f32, tag="a2r", name="a2r")
                a2r_t[bi] = a2r
                for g in range(3):
                    nj = min(3, NB - 3 * g)
                    for j in range(nj):
                        nc.tensor.matmul(
                            a2r[32 * g:32 * g + 32, 0:blk],
                            selsb[:, j, :],
                            a2tmp[:, 0:blk, 3 * g + j],
                            start=(j == 0), stop=(j == nj - 1),
                        )

            def emit_wscan(bi):
                tb0, tb1 = blocks[bi]
                blk = tb1 - tb0
                # W1[t] = (a2s[t-1] + W1[t-1]) * p   (hardware scan)
                nc.vector.tensor_tensor_scan(
                    W1[:, tb0 + 1:tb1 + 1], a2s[:, tb0:tb1], pcsb[:, 0:blk],
                    W1[:, tb0:tb0 + 1], Alu.add, Alu.mult,
                )
                nc.vector.tensor_tensor_scan(
                    W2[:, tb0 + 1:tb1 + 1], W1[:, tb0:tb1], pcsb[:, 0:blk],
                    W2[:, tb0:tb0 + 1], Alu.add, Alu.mult,
                )
                # u2/20 = W1 + W2 -> layer-2 column of Up, lagged by LAG
                nc.gpsimd.tensor_tensor(
                    Up[0:84, 32, tb0 + LAG:tb1 + LAG],
                    W1[:, tb0 + 1:tb1 + 1], W2[:, tb0 + 1:tb1 + 1], Alu.add,
                )

            # ---- fused scan: layer-1 at step tau, layer-2 at tau-LAG ----
            MW = M.ap[0][0]          # mega-tile row stride (elements)
            MOFF = M.offset

            def m_ap(off, dims, parts=PMAX):
                return bass.AP(M.tensor, MOFF + off, [[MW, parts]] + dims)

            def emit_uchunk(g0):
                if g0 >= T:
                    return
                g1 = min(g0 + UCHUNK, T)
                w = g1 - g0
                # Up[p, j, t] = A1[p, j] * c20[t] — broadcast outer product
                nc.gpsimd.tensor_tensor(
                    Up[:, 0:32, g0:g1],
                    A1[:].unsqueeze(2).broadcast_to([PMAX, 32, w]),
                    c20sb[:, g0:g1].unsqueeze(1).broadcast_to([PMAX, 32, w]),
                    Alu.mult,
                )

            emit_uchunk(0)
            emit_uchunk(UCHUNK)

            for tau in range(TF):
                if tau < T:
                    pp = m_ap(SW, [[1, NCOL]])
                    rr = m_ap(SW + NCOL, [[1, NCOL]])
                    w0, off = NCOL, 0
                else:
                    # tail: layer-1 finished, only column 32 is live
                    pp = m_ap(SW + 32, [[1, 1]])
                    rr = m_ap(SW + NCOL + 32, [[1, 1]])
                    w0, off = 1, 32
                # P = q*P + s_{tau-1}
                nc.vector.scalar_tensor_tensor(
                    pp, pp, q, m_ap(tau * NCOL + off, [[1, w0]]),
                    Alu.mult, Alu.add,
                )
                # R = q*R + P
                nc.vector.scalar_tensor_tensor(
                    rr, rr, q, pp, Alu.mult, Alu.add,
                )
                # s_{tau} = (R + 0.5) <= u/20
                nc.vector.scalar_tensor_tensor(
                    m_ap((tau + 1) * NCOL + off, [[1, w0]]),
                    rr, 0.5,
                    bass.AP(Up.tensor, Up.offset + off * TF + tau,
                            [[UW, PMAX], [TF, w0]]),
                    Alu.add, Alu.is_le,
                )

                if tau == OSPLIT - 1 + LAG and tau >= T:
                    nc.scalar.activation(
                        ostage[:, 0:OSPLIT],
                        m_ap((LAG + 1) * NCOL + 32, [[NCOL, OSPLIT]],
                             parts=84),
                        Act.Copy,
                    )
                    nc.sync.dma_start(out_d[:, 0:OSPLIT], ostage[:, 0:OSPLIT])
                if tau < T:
                    # PE fc2 block launch (waits on spike history via sems)
                    if tau in blk_at:
                        bi, tb0, tb1 = blk_at[tau]
                        blk = tb1 - tb0
                        a2ps = ps2.tile([NO2, B, NB], f32, tag="a2ps",
                                        name="a2ps")
                        a2ps_t[bi] = a2ps
                        for c in range(4):
                            nc.tensor.matmul(
                                a2ps[:, 0:blk, :],
                                w2sb[:, c, :],
                                m_ap((tb0 + 1) * NCOL + c * NB,
                                     [[NCOL, blk], [1, NB]]),
                                start=(c == 0), stop=(c == 3),
                            )
                    # threshold chunks stay 2 ahead of the scan
                    if tau % UCHUNK == 0 and tau + 2 * UCHUNK < T + UCHUNK:
                        emit_uchunk(tau + 2 * UCHUNK)
                    if tau == OSPLIT - 1 + LAG:
                        # early output half overlaps the remaining tail
                        nc.scalar.activation(
                            ostage[:, 0:OSPLIT],
                            m_ap((LAG + 1) * NCOL + 32, [[NCOL, OSPLIT]],
                                 parts=84),
                            Act.Copy,
                        )
                        nc.sync.dma_start(out_d[:, 0:OSPLIT],
                                          ostage[:, 0:OSPLIT])
                    for bi, which in evac_at.get(tau, []):
                        emit_evac(bi, which)
                        if which == 1:
                            emit_restack(bi)
                    # DVE stream: delayed PSP scans
                    for bi in wscan_at.get(tau, []):
                        emit_wscan(bi)

            # ---- output: remaining layer-2 spikes ----
            nc.scalar.activation(
                ostage[:, OSPLIT:T],
                m_ap((OSPLIT + LAG + 1) * NCOL + 32, [[NCOL, T - OSPLIT]],
                     parts=84),
                Act.Copy,
            )
            nc.sync.dma_start(out_d[:, OSPLIT:T], ostage[:, OSPLIT:T])

    nc.compile()
    return nc


def _host_inputs(input, w1, w2):
    f32 = np.float32
    q, p, k2, c20 = _consts()
    flat = np.ascontiguousarray(input.reshape(64, -1).astype(f32))
    # fTp[p, k, n] = flat[n, k*128+p]
    fTp = np.ascontiguousarray(
        flat.T.reshape(KT, 128, 64).transpose(1, 0, 2))  # (128, KT, 64)
    # w1p[p, k*410+o] = w1[o, k*128+p]
    w1T = w1.astype(f32).T.reshape(KT, 128, NO1)          # (k, p, o)
    w1p = np.ascontiguousarray(
        w1T.transpose(1, 0, 2).reshape(128, KT * NO1))
    w2p = np.zeros((PMAX, 4, NO2), f32)
    for c in range(4):
        w2p[0:MC[c], c, :] = w2.astype(f32)[:, OFF[c]:OFF[c] + MC[c]].T
    pconst = np.full((84, B), p, f32)
    eye8 = np.eye(NB, dtype=f32)
    sel32 = np.zeros((NO2, 3, 32), f32)
    for j in range(3):
        for o2 in range(NO2):
            sel32[o2, j, 10 * j + o2] = 1.0
    c20rep = np.broadcast_to(c20, (PMAX, T)).copy()
    return fTp, w1p, w2p, pconst, eye8, sel32, c20rep


def kernel(input, w1, w2):
    from concourse.bass_utils import run_bass_kernel_spmd

    if "nc" not in _CACHE:
        _CACHE["nc"] = build()
    nc = _CACHE["nc"]

    fTp, w1p, w2p, pconst, eye8, sel32, c20rep = _host_inputs(input, w1, w2)
    in_maps = []
    for core in range(8):
        fTc = fTp[:, :, core * NB:(core + 1) * NB].reshape(128, KT * NB)
        in_maps.append({
            "fTp": np.ascontiguousarray(fTc),
            "w1p": w1p,
            "w2p": w2p,
            "pconst": pconst,
            "eye8": eye8,
            "sel32": sel32,
            "c20rep": c20rep,
        })
    res = run_bass_kernel_spmd(nc, in_maps, core_ids=list(range(8)))
    # row r = 32*g + 10*j + o2 holds batch n = 3*g + j
    rows = np.array([32 * (n // 3) + 10 * (n % 3) + np.arange(NO2)
                     for n in range(NB)])          # (8, 10)
    full = np.zeros((64, NO2, T), np.float32)
    for core in range(8):
        o = res.results[core]["out"]               # (84, T)
        full[core * NB:(core + 1) * NB] = o[rows]
    return full


# revision 17
# speedup vs baseline: 1.1452x; 1.0135x over previous
"""Trainium2 Bass kernel for the SLAYER-style 2-layer spiking MLP.

Reference computation (per batch element n):
    flat   = input.reshape(64, 3072)
    a1     = flat @ w1.T                      (constant over time)
    u1[t]  = a1 * c[t]          where c = cumsum(srm kernel)  (PSP of a
             time-constant input is just a ramp scale)
    s1     = spike_scan(u1)     sequential threshold w/ refractory feedback
    a2[t]  = w2 @ s1[:, t]
    u2     = psp(a2)            (true temporal conv, srm kernel)
    out    = spike_scan(u2)

Refractory feedback is an exact order-2 IIR (kernel rk[d] = -20 d e^{1-d}):
    P[t] = q*P[t-1] + s[t-1];  R[t] = q*R[t-1] + P[t]   (q = e^-1)
    spike:  (R + 0.5) <= u/20
Each fused scan step is 3 scalar_tensor_tensor DVE ops over a [104, 33]
tile holding both layers (layer 2 rides along lagged LAG steps).

Schedule highlights vs the naive version:
  * w1 / flat are host-packed so each DMA moves long contiguous
    per-partition lines (128 descriptors instead of 3072).
  * The u1/20 threshold rows are produced per-step on the otherwise-idle
    Scalar engine (activation Copy, scale=c20[t]) instead of one huge
    broadcast-AP DVE op that serialized the whole prologue.
  * PSUM evacuations ride the scalar stream at delayed slots so they
    never block threshold-row production.
  * fc2 re-stack uses a shared eye(10) stationary with per-n PSUM
    partition-offset writes (1 LDWEIGHTS instead of 8 full selector
    loads per block).

Sharding: data-parallel over batch, 8 elements per core, weights replicated.
"""

import numpy as np

NB = 8            # batch elements per core
T = 100           # timesteps
B = 16            # pipeline block size
LAG = 32          # layer-2 ride-along lag (>= block + pipeline latency)
TF = T + LAG      # fused scan steps
NCOL = 33         # 32 layer-1 columns (4 chunks x 8 batch) + 1 layer-2 column
PMAX = 104        # padded partition count per o-chunk
MC = [103, 103, 102, 102]      # o-chunk sizes (sum = 410)
OFF = [0, 103, 206, 308]
KT = 24           # 3072 / 128 k-tiles
KCS = [2, 6, 8, 8]  # k-tiles per w1 DMA chunk (small first chunk)
NO1 = 410
NO2 = 10
DELAY = 16        # DVE-stream slots after block end before W1/W2 scans
EVAC1_DELAY = 8   # stream slots after block end before PSUM evac 1
EVAC2_DELAY = 15  # (selector MMs are emitted right after evac1)
UCHUNK = 12       # threshold-surface timesteps per gpsimd instruction
OSPLIT = 88       # output columns staged/DMA'd early, overlapping the tail

_CACHE = {}


def _consts():
    q = float(np.float32(np.exp(-1.0)))          # refractory ratio
    p = float(np.float32(np.exp(-0.1)))          # SRM ratio
    k2 = float(np.float32(np.exp(1.0) / 200.0))  # a2 pre-scale: u2/20 = sum
    t = np.arange(T, dtype=np.float64)
    srm = (t / 10.0) * np.exp(1.0 - t / 10.0)
    c20 = (np.cumsum(srm) / 20.0).astype(np.float32)
    return q, p, k2, c20


def build():
    import concourse.bass as bass
    import concourse.bacc as bacc
    import concourse.mybir as mybir
    from concourse import tile

    f32 = mybir.dt.float32
    Alu = mybir.AluOpType
    Act = mybir.ActivationFunctionType
    q, p, k2, c20 = _consts()

    nc = bacc.Bacc("TRN2", target_bir_lowering=False, debug=False, num_devices=8)

    w1p_d = nc.dram_tensor("w1p", [128, KT * NO1], f32, kind="ExternalInput")
    fTp_d = nc.dram_tensor("fTp", [128, KT * NB], f32, kind="ExternalInput")
    w2p_d = nc.dram_tensor("w2p", [PMAX, 4, NO2], f32, kind="ExternalInput")
    pc_d = nc.dram_tensor("pconst", [84, B], f32, kind="ExternalInput")
    eye_d = nc.dram_tensor("eye8", [NB, NB], f32, kind="ExternalInput")
    sel_d = nc.dram_tensor("sel32", [NO2, 3, 32], f32, kind="ExternalInput")
    c20_d = nc.dram_tensor("c20rep", [PMAX, T], f32, kind="ExternalInput")
    out_d = nc.dram_tensor("out", [84, T], f32, kind="ExternalOutput")

    with tile.TileContext(nc) as tc:
        with (
            tc.tile_pool(name="pers", bufs=1) as pool,
            tc.tile_pool(name="ps1", bufs=1, space="PSUM") as ps1,
            tc.tile_pool(name="ps2", bufs=2, space="PSUM") as ps2,
        ):
            w1sb = pool.tile([128, KT, NO1], f32, tag="w1sb")
            fTsb = pool.tile([128, KT, NB], f32, tag="fTsb")
            w2sb = pool.tile([PMAX, 4, NO2], f32, tag="w2sb")
            pcsb = pool.tile([84, B], f32, tag="pcsb")
            eyesb = pool.tile([NB, NB], f32, tag="eyesb")
            selsb = pool.tile([NO2, 3, 32], f32, tag="selsb")
            a1rsb = pool.tile([NB, NO1], f32, tag="a1rsb")
            A1 = pool.tile([PMAX, 32], f32, tag="A1")
            Up = pool.tile([PMAX, NCOL, TF], f32, tag="Up")
            c20sb = pool.tile([PMAX, T], f32, tag="c20sb")
            # mega-tile: spike history S (TF+1 slots of NCOL) followed by the
            # IIR state [P(NCOL) | R(NCOL)].
            SW = (TF + 1) * NCOL
            M = pool.tile([PMAX, SW + 2 * NCOL], f32, tag="M")
            a2tmp = pool.tile([NO2, B, NB], f32, tag="a2tmp")
            a2s = pool.tile([84, T + 1], f32, tag="a2s")
            W1 = pool.tile([84, T + 1], f32, tag="W1")
            W2 = pool.tile([84, T + 1], f32, tag="W2")
            ostage = pool.tile([84, T], f32, tag="ostage")

            # ---- input DMAs: flat first (gates fc1's stationary), then w1
            # chunks (small chunk 0 so fc1 starts early); constants on sync ----
            nc.gpsimd.dma_start(fTsb[:], fTp_d[:])
            ko = 0
            for kc in KCS:
                nc.gpsimd.dma_start(
                    w1sb[:, ko:ko + kc, :],
                    w1p_d[:, ko * NO1:(ko + kc) * NO1],
                )
                ko += kc
            nc.sync.dma_start(w2sb[:], w2p_d[:])
            nc.sync.dma_start(pcsb[:], pc_d[:])
            nc.sync.dma_start(eyesb[:], eye_d[:])
            nc.sync.dma_start(selsb[:], sel_d[:])
            nc.sync.dma_start(c20sb[:], c20_d[:])

            # ---- state init (rides during DMA) ----
            nc.vector.memset(M[:, 0:NCOL], 0.0)            # S slot 0
            nc.vector.memset(M[:, SW:SW + 2 * NCOL], 0.0)  # P | R
            nc.vector.memset(A1[:], 0.0)
            # layer-2 u column (incl. garbage rows) — NaN insurance
            UW = Up.ap[0][0]
            nc.gpsimd.memset(Up[:, 32, :], 0.0)
            nc.gpsimd.memset(a2s[:, 0:1], 0.0)
            nc.gpsimd.memset(W1[:, 0:1], 0.0)
            nc.gpsimd.memset(W2[:, 0:1], 0.0)

            # ---- PE warmup: ramp the tensor-engine clock on w2sb while the
            # w1 DMA is still in flight (results are discarded) ----
            wrm = ps1.tile([NO2, 40], f32, tag="wrm", name="wrm")
            for _ in range(5):
                nc.tensor.matmul(wrm[:], w2sb[:, 0, :], w2sb[:],
                                 start=True, stop=True)

            # ---- fc1: a1row[n, o] = flat @ w1.T, accumulated over k ----
            a1row = ps1.tile([NB, NO1], f32, tag="a1row", name="a1row")
            for k in range(KT):
                nc.tensor.matmul(
                    a1row[:], fTsb[:, k, :], w1sb[:, k, :],
                    start=(k == 0), stop=(k == KT - 1),
                )
            nc.vector.tensor_copy(a1rsb[:], a1row[:])
            # transpose to scan layout: A1[o_chunk, (c, n)]
            for c in range(4):
                a1tp = ps1.tile([PMAX, NB], f32, tag=f"a1tp{c % 2}", name="a1tp")
                nc.tensor.transpose(
                    a1tp[0:MC[c], :],
                    a1rsb[0:NB, OFF[c]:OFF[c] + MC[c]],
                    eyesb[:],
                )
                nc.vector.tensor_copy(
                    A1[0:MC[c], c * NB:(c + 1) * NB], a1tp[0:MC[c], :]
                )

            # ---- block/scalar-stream schedules ----
            blocks = []
            for b in range((T + B - 1) // B):
                blocks.append((b * B, min((b + 1) * B, T)))
            blk_at = {tb1 - 1: (bi, tb0, tb1)
                      for bi, (tb0, tb1) in enumerate(blocks)}
            wscan_at = {}
            for bi, (tb0, tb1) in enumerate(blocks):
                wscan_at.setdefault(min(tb1 - 1 + DELAY, T - 1), []).append(bi)
            evac_at = {}
            for bi, (tb0, tb1) in enumerate(blocks):
                evac_at.setdefault(min(tb1 - 1 + EVAC1_DELAY, T - 1), []).append(
                    (bi, 1))
                evac_at.setdefault(min(tb1 - 1 + EVAC2_DELAY, T - 1), []).append(
                    (bi, 2))

            a2ps_t = {}
            a2r_t = {}

            def emit_evac(bi, which):
                tb0, tb1 = blocks[bi]
                blk = tb1 - tb0
                if which == 1:
                    nc.scalar.activation(
                        a2tmp[:, 0:blk, :], a2ps_t[bi][:, 0:blk, :],
                        Act.Copy, scale=k2,
                    )
                else:
                    nc.scalar.activation(
                        a2s[:, tb0 + 1:tb1 + 1], a2r_t[bi][0:84, 0:blk],
                        Act.Copy,
                    )

            def emit_restack(bi):
                tb0, tb1 = blocks[bi]
                blk = tb1 - tb0
                # re-stack to gapped rows (r = 32g+10j+o2): shared shifted
                # identities, PSUM partition bases 0/32/64
                a2r = ps2.tile([96, B], f32, tag="a2r", name="a2r")
                a2r_t[bi] = a2r
                for g in range(3):
                    nj = min(3, NB - 3 * g)
                    for j in range(nj):
                        nc.tensor.matmul(
                            a2r[32 * g:32 * g + 32, 0:blk],
                            selsb[:, j, :],
                            a2tmp[:, 0:blk, 3 * g + j],
                            start=(j == 0), stop=(j == nj - 1),
                        )

            def emit_wscan(bi):
                tb0, tb1 = blocks[bi]
                blk = tb1 - tb0
                # W1[t] = (a2s[t-1] + W1[t-1]) * p   (hardware scan)
                nc.vector.tensor_tensor_scan(
                    W1[:, tb0 + 1:tb1 + 1], a2s[:, tb0:tb1], pcsb[:, 0:blk],
                    W1[:, tb0:tb0 + 1], Alu.add, Alu.mult,
                )
                nc.vector.tensor_tensor_scan(
                    W2[:, tb0 + 1:tb1 + 1], W1[:, tb0:tb1], pcsb[:, 0:blk],
                    W2[:, tb0:tb0 + 1], Alu.add, Alu.mult,
                )
                # u2/20 = W1 + W2 -> layer-2 column of Up, lagged by LAG
                nc.gpsimd.tensor_tensor(
                    Up[0:84, 32, tb0 + LAG:tb1 + LAG],
                    W1[:, tb0 + 1:tb1 + 1], W2[:, tb0 + 1:tb1 + 1], Alu.add,
                )

            # ---- fused scan: layer-1 at step tau, layer-2 at tau-LAG ----
            MW = M.ap[0][0]          # mega-tile row stride (elements)
            MOFF = M.offset

            def m_ap(off, dims, parts=PMAX):
                return bass.AP(M.tensor, MOFF + off, [[MW, parts]] + dims)

            def emit_uchunk(g0):
                if g0 >= T:
                    return
                g1 = min(g0 + UCHUNK, T)
                w = g1 - g0
                # Up[p, j, t] = A1[p, j] * c20[t] — broadcast outer product
                nc.gpsimd.tensor_tensor(
                    Up[:, 0:32, g0:g1],
                    A1[:].unsqueeze(2).broadcast_to([PMAX, 32, w]),
                    c20sb[:, g0:g1].unsqueeze(1).broadcast_to([PMAX, 32, w]),
                    Alu.mult,
                )

            # small lead-in chunk so op3(0) unblocks quickly, then full ones
            g1 = 2
            nc.gpsimd.tensor_tensor(
                Up[:, 0:32, 0:g1],
                A1[:].unsqueeze(2).broadcast_to([PMAX, 32, g1]),
                c20sb[:, 0:g1].unsqueeze(1).broadcast_to([PMAX, 32, g1]),
                Alu.mult,
            )
            nc.gpsimd.tensor_tensor(
                Up[:, 0:32, g1:UCHUNK],
                A1[:].unsqueeze(2).broadcast_to([PMAX, 32, UCHUNK - g1]),
                c20sb[:, g1:UCHUNK].unsqueeze(1).broadcast_to(
                    [PMAX, 32, UCHUNK - g1]),
                Alu.mult,
            )
            emit_uchunk(UCHUNK)

            for tau in range(TF):
                if tau < T:
                    pp = m_ap(SW, [[1, NCOL]])
                    rr = m_ap(SW + NCOL, [[1, NCOL]])
                    w0, off = NCOL, 0
                else:
                    # tail: layer-1 finished, only column 32 is live
                    pp = m_ap(SW + 32, [[1, 1]])
                    rr = m_ap(SW + NCOL + 32, [[1, 1]])
                    w0, off = 1, 32
                # P = q*P + s_{tau-1}
                nc.vector.scalar_tensor_tensor(
                    pp, pp, q, m_ap(tau * NCOL + off, [[1, w0]]),
                    Alu.mult, Alu.add,
                )
                # R = q*R + P
                nc.vector.scalar_tensor_tensor(
                    rr, rr, q, pp, Alu.mult, Alu.add,
                )
                # s_{tau} = (R + 0.5) <= u/20
                nc.vector.scalar_tensor_tensor(
                    m_ap((tau + 1) * NCOL + off, [[1, w0]]),
                    rr, 0.5,
                    bass.AP(Up.tensor, Up.offset + off * TF + tau,
                            [[UW, PMAX], [TF, w0]]),
                    Alu.add, Alu.is_le,
                )

                if tau == OSPLIT - 1 + LAG and tau >= T:
                    nc.scalar.activation(
                        ostage[:, 0:OSPLIT],
                        m_ap((LAG + 1) * NCOL + 32, [[NCOL, OSPLIT]],
                             parts=84),
                        Act.Copy,
                    )
                    nc.sync.dma_start(out_d[:, 0:OSPLIT], ostage[:, 0:OSPLIT])
                if tau < T:
                    # PE fc2 block launch (waits on spike history via sems)
                    if tau in blk_at:
                        bi, tb0, tb1 = blk_at[tau]
                        blk = tb1 - tb0
                        a2ps = ps2.tile([NO2, B, NB], f32, tag="a2ps",
                                        name="a2ps")
                        a2ps_t[bi] = a2ps
                        for c in range(4):
                            nc.tensor.matmul(
                                a2ps[:, 0:blk, :],
                                w2sb[:, c, :],
                                m_ap((tb0 + 1) * NCOL + c * NB,
                                     [[NCOL, blk], [1, NB]]),
                                start=(c == 0), stop=(c == 3),
                            )
                    # threshold chunks stay 2 ahead of the scan
                    if tau % UCHUNK == 0 and tau + 2 * UCHUNK < T + UCHUNK:
                        emit_uchunk(tau + 2 * UCHUNK)
                    if tau == OSPLIT - 1 + LAG:
                        # early output half overlaps the remaining tail
                        nc.scalar.activation(
                            ostage[:, 0:OSPLIT],
                            m_ap((LAG + 1) * NCOL + 32, [[NCOL, OSPLIT]],
                                 parts=84),
                            Act.Copy,
                        )
                        nc.sync.dma_start(out_d[:, 0:OSPLIT],
                                          ostage[:, 0:OSPLIT])
                    for bi, which in evac_at.get(tau, []):
                        emit_evac(bi, which)
                        if which == 1:
                            emit_restack(bi)
                    # DVE stream: delayed PSP scans
                    for bi in wscan_at.get(tau, []):
                        emit_wscan(bi)

            # ---- output: remaining layer-2 spikes ----
            nc.scalar.activation(
                ostage[:, OSPLIT:T],
                m_ap((OSPLIT + LAG + 1) * NCOL + 32, [[NCOL, T - OSPLIT]],
                     parts=84),
                Act.Copy,
            )
            nc.sync.dma_start(out_d[:, OSPLIT:T], ostage[:, OSPLIT:T])

    nc.compile()
    return nc


def _host_inputs(input, w1, w2):
    f32 = np.float32
    q, p, k2, c20 = _consts()
    flat = np.ascontiguousarray(input.reshape(64, -1).astype(f32))
    # fTp[p, k, n] = flat[n, k*128+p]
    fTp = np.ascontiguousarray(
        flat.T.reshape(KT, 128, 64).transpose(1, 0, 2))  # (128, KT, 64)
    # w1p[p, k*410+o] = w1[o, k*128+p]
    w1T = w1.astype(f32).T.reshape(KT, 128, NO1)          # (k, p, o)
    w1p = np.ascontiguousarray(
        w1T.transpose(1, 0, 2).reshape(128, KT * NO1))
    w2p = np.zeros((PMAX, 4, NO2), f32)
    for c in range(4):
        w2p[0:MC[c], c, :] = w2.astype(f32)[:, OFF[c]:OFF[c] + MC[c]].T
    pconst = np.full((84, B), p, f32)
    eye8 = np.eye(NB, dtype=f32)
    sel32 = np.zeros((NO2, 3, 32), f32)
    for j in range(3):
        for o2 in range(NO2):
            sel32[o2, j, 10 * j + o2] = 1.0
    c20rep = np.broadcast_to(c20, (PMAX, T)).copy()
    return fTp, w1p, w2p, pconst, eye8, sel32, c20rep


def kernel(input, w1, w2):
    from concourse.bass_utils import run_bass_kernel_spmd

    if "nc" not in _CACHE:
        _CACHE["nc"] = build()
    nc = _CACHE["nc"]

    fTp, w1p, w2p, pconst, eye8, sel32, c20rep = _host_inputs(input, w1, w2)
    in_maps = []
    for core in range(8):
        fTc = fTp[:, :, core * NB:(core + 1) * NB].reshape(128, KT * NB)
        in_maps.append({
            "fTp": np.ascontiguousarray(fTc),
            "w1p": w1p,
            "w2p": w2p,
            "pconst": pconst,
            "eye8": eye8,
            "sel32": sel32,
            "c20rep": c20rep,
        })
    res = run_bass_kernel_spmd(nc, in_maps, core_ids=list(range(8)))
    # row r = 32*g + 10*j + o2 holds batch n = 3*g + j
    rows = np.array([32 * (n // 3) + 10 * (n % 3) + np.arange(NO2)
                     for n in range(NB)])          # (8, 10)
    full = np.zeros((64, NO2, T), np.float32)
    for core in range(8):
        o = res.results[core]["out"]               # (84, T)
        full[core * NB:(core + 1) * NB] = o[rows]
    return full


# revision 18
# speedup vs baseline: 1.1577x; 1.0109x over previous
"""Trainium2 Bass kernel for the SLAYER-style 2-layer spiking MLP.

Reference computation (per batch element n):
    flat   = input.reshape(64, 3072)
    a1     = flat @ w1.T                      (constant over time)
    u1[t]  = a1 * c[t]          where c = cumsum(srm kernel)  (PSP of a
             time-constant input is just a ramp scale)
    s1     = spike_scan(u1)     sequential threshold w/ refractory feedback
    a2[t]  = w2 @ s1[:, t]
    u2     = psp(a2)            (true temporal conv, srm kernel)
    out    = spike_scan(u2)

Refractory feedback is an exact order-2 IIR (kernel rk[d] = -20 d e^{1-d}):
    P[t] = q*P[t-1] + s[t-1];  R[t] = q*R[t-1] + P[t]   (q = e^-1)
    spike:  (R + 0.5) <= u/20
Each fused scan step is 3 scalar_tensor_tensor DVE ops over a [104, 33]
tile holding both layers (layer 2 rides along lagged LAG steps).

Schedule highlights vs the naive version:
  * w1 / flat are host-packed so each DMA moves long contiguous
    per-partition lines (128 descriptors instead of 3072).
  * The u1/20 threshold rows are produced per-step on the otherwise-idle
    Scalar engine (activation Copy, scale=c20[t]) instead of one huge
    broadcast-AP DVE op that serialized the whole prologue.
  * PSUM evacuations ride the scalar stream at delayed slots so they
    never block threshold-row production.
  * fc2 re-stack uses a shared eye(10) stationary with per-n PSUM
    partition-offset writes (1 LDWEIGHTS instead of 8 full selector
    loads per block).

Sharding: data-parallel over batch, 8 elements per core, weights replicated.
"""

import numpy as np

NB = 8            # batch elements per core
T = 100           # timesteps
B = 16            # pipeline block size
LAG = 32          # layer-2 ride-along lag (>= block + pipeline latency)
TF = T + LAG      # fused scan steps
NCOL = 33         # 32 layer-1 columns (4 chunks x 8 batch) + 1 layer-2 column
PMAX = 104        # padded partition count per o-chunk
MC = [103, 103, 102, 102]      # o-chunk sizes (sum = 410)
OFF = [0, 103, 206, 308]
KT = 24           # 3072 / 128 k-tiles
KCS = [2, 6, 8, 8]  # k-tiles per w1 DMA chunk (small first chunk)
NO1 = 410
NO2 = 10
DELAY = 16        # DVE-stream slots after block end before W1/W2 scans
EVAC1_DELAY = 8   # stream slots after block end before PSUM evac 1
EVAC2_DELAY = 15  # (selector MMs are emitted right after evac1)
UCHUNK = 12       # threshold-surface timesteps per gpsimd instruction
OSPLIT = 88       # output columns staged/DMA'd early, overlapping the tail

_CACHE = {}


def _consts():
    q = float(np.float32(np.exp(-1.0)))          # refractory ratio
    p = float(np.float32(np.exp(-0.1)))          # SRM ratio
    k2 = float(np.float32(np.exp(1.0) / 200.0))  # a2 pre-scale: u2/20 = sum
    t = np.arange(T, dtype=np.float64)
    srm = (t / 10.0) * np.exp(1.0 - t / 10.0)
    c20 = (np.cumsum(srm) / 20.0).astype(np.float32)
    return q, p, k2, c20


def build():
    import concourse.bass as bass
    import concourse.bacc as bacc
    import concourse.mybir as mybir
    from concourse import tile

    f32 = mybir.dt.float32
    Alu = mybir.AluOpType
    Act = mybir.ActivationFunctionType
    q, p, k2, c20 = _consts()

    nc = bacc.Bacc("TRN2", target_bir_lowering=False, debug=False, num_devices=8)

    w1p_d = nc.dram_tensor("w1p", [128, KT * NO1], f32, kind="ExternalInput")
    fTp_d = nc.dram_tensor("fTp", [128, KT * NB], f32, kind="ExternalInput")
    w2p_d = nc.dram_tensor("w2p", [PMAX, 4, NO2], f32, kind="ExternalInput")
    pc_d = nc.dram_tensor("pconst", [84, B], f32, kind="ExternalInput")
    eye_d = nc.dram_tensor("eye8", [NB, NB], f32, kind="ExternalInput")
    sel_d = nc.dram_tensor("sel32", [NO2, 3, 32], f32, kind="ExternalInput")
    c20_d = nc.dram_tensor("c20rep", [PMAX, T], f32, kind="ExternalInput")
    out_d = nc.dram_tensor("out", [84, T], f32, kind="ExternalOutput")

    with tile.TileContext(nc) as tc:
        with (
            tc.tile_pool(name="pers", bufs=1) as pool,
            tc.tile_pool(name="ps1", bufs=1, space="PSUM") as ps1,
            tc.tile_pool(name="ps2", bufs=2, space="PSUM") as ps2,
        ):
            w1sb = pool.tile([128, KT, NO1], f32, tag="w1sb")
            fTsb = pool.tile([128, KT, NB], f32, tag="fTsb")
            w2sb = pool.tile([PMAX, 4, NO2], f32, tag="w2sb")
            pcsb = pool.tile([84, B], f32, tag="pcsb")
            eyesb = pool.tile([NB, NB], f32, tag="eyesb")
            selsb = pool.tile([NO2, 3, 32], f32, tag="selsb")
            a1rsb = pool.tile([NB, NO1], f32, tag="a1rsb")
            A1 = pool.tile([PMAX, 32], f32, tag="A1")
            Up = pool.tile([PMAX, NCOL, TF], f32, tag="Up")
            c20sb = pool.tile([PMAX, T], f32, tag="c20sb")
            # mega-tile: spike history S (TF+1 slots of NCOL) followed by the
            # IIR state [P(NCOL) | R(NCOL)].
            SW = (TF + 1) * NCOL
            M = pool.tile([PMAX, SW + 2 * NCOL], f32, tag="M")
            a2tmp = pool.tile([NO2, B, NB], f32, tag="a2tmp")
            a2s = pool.tile([84, T + 1], f32, tag="a2s")
            W1 = pool.tile([84, T + 1], f32, tag="W1")
            W2 = pool.tile([84, T + 1], f32, tag="W2")
            ostage = pool.tile([84, T], f32, tag="ostage")

            # ---- input DMAs: flat first (gates fc1's stationary), then w1
            # chunks (small chunk 0 so fc1 starts early); constants on sync ----
            nc.gpsimd.dma_start(fTsb[:], fTp_d[:])
            ko = 0
            for kc in KCS:
                nc.gpsimd.dma_start(
                    w1sb[:, ko:ko + kc, :],
                    w1p_d[:, ko * NO1:(ko + kc) * NO1],
                )
                ko += kc
            nc.sync.dma_start(w2sb[:], w2p_d[:])
            nc.sync.dma_start(pcsb[:], pc_d[:])
            nc.sync.dma_start(eyesb[:], eye_d[:])
            nc.sync.dma_start(selsb[:], sel_d[:])
            nc.sync.dma_start(c20sb[:], c20_d[:])

            # ---- state init (rides during DMA) ----
            nc.vector.memset(M[:, 0:NCOL], 0.0)            # S slot 0
            nc.vector.memset(M[:, SW:SW + 2 * NCOL], 0.0)  # P | R
            nc.vector.memset(A1[:], 0.0)
            # layer-2 u column (incl. garbage rows) — NaN insurance
            UW = Up.ap[0][0]
            nc.gpsimd.memset(Up[:, 32, :], 0.0)
            nc.gpsimd.memset(a2s[:, 0:1], 0.0)
            nc.gpsimd.memset(W1[:, 0:1], 0.0)
            nc.gpsimd.memset(W2[:, 0:1], 0.0)

            # ---- PE warmup: ramp the tensor-engine clock on w2sb while the
            # w1 DMA is still in flight (results are discarded) ----
            wrm = ps1.tile([NO2, 40], f32, tag="wrm", name="wrm")
            for _ in range(16):
                nc.tensor.matmul(wrm[:], w2sb[:, 0, :], w2sb[:],
                                 start=True, stop=True)

            # ---- fc1: a1row[n, o] = flat @ w1.T, accumulated over k ----
            a1row = ps1.tile([NB, NO1], f32, tag="a1row", name="a1row")
            for k in range(KT):
                nc.tensor.matmul(
                    a1row[:], fTsb[:, k, :], w1sb[:, k, :],
                    start=(k == 0), stop=(k == KT - 1),
                )
            nc.vector.tensor_copy(a1rsb[:], a1row[:])
            # transpose to scan layout: A1[o_chunk, (c, n)]
            for c in range(4):
                a1tp = ps1.tile([PMAX, NB], f32, tag=f"a1tp{c % 2}", name="a1tp")
                nc.tensor.transpose(
                    a1tp[0:MC[c], :],
                    a1rsb[0:NB, OFF[c]:OFF[c] + MC[c]],
                    eyesb[:],
                )
                nc.vector.tensor_copy(
                    A1[0:MC[c], c * NB:(c + 1) * NB], a1tp[0:MC[c], :]
                )

            # ---- block/scalar-stream schedules ----
            blocks = []
            for b in range((T + B - 1) // B):
                blocks.append((b * B, min((b + 1) * B, T)))
            blk_at = {tb1 - 1: (bi, tb0, tb1)
                      for bi, (tb0, tb1) in enumerate(blocks)}
            wscan_at = {}
            for bi, (tb0, tb1) in enumerate(blocks):
                wscan_at.setdefault(min(tb1 - 1 + DELAY, T - 1), []).append(bi)
            evac_at = {}
            for bi, (tb0, tb1) in enumerate(blocks):
                evac_at.setdefault(min(tb1 - 1 + EVAC1_DELAY, T - 1), []).append(
                    (bi, 1))
                evac_at.setdefault(min(tb1 - 1 + EVAC2_DELAY, T - 1), []).append(
                    (bi, 2))

            a2ps_t = {}
            a2r_t = {}

            def emit_evac(bi, which):
                tb0, tb1 = blocks[bi]
                blk = tb1 - tb0
                if which == 1:
                    nc.scalar.activation(
                        a2tmp[:, 0:blk, :], a2ps_t[bi][:, 0:blk, :],
                        Act.Copy, scale=k2,
                    )
                else:
                    nc.scalar.activation(
                        a2s[:, tb0 + 1:tb1 + 1], a2r_t[bi][0:84, 0:blk],
                        Act.Copy,
                    )

            def emit_restack(bi):
                tb0, tb1 = blocks[bi]
                blk = tb1 - tb0
                # re-stack to gapped rows (r = 32g+10j+o2): shared shifted
                # identities, PSUM partition bases 0/32/64
                a2r = ps2.tile([96, B], f32, tag="a2r", name="a2r")
                a2r_t[bi] = a2r
                for g in range(3):
                    nj = min(3, NB - 3 * g)
                    for j in range(nj):
                        nc.tensor.matmul(
                            a2r[32 * g:32 * g + 32, 0:blk],
                            selsb[:, j, :],
                            a2tmp[:, 0:blk, 3 * g + j],
                            start=(j == 0), stop=(j == nj - 1),
                        )

            def emit_wscan(bi):
                tb0, tb1 = blocks[bi]
                blk = tb1 - tb0
                # W1[t] = (a2s[t-1] + W1[t-1]) * p   (hardware scan)
                nc.vector.tensor_tensor_scan(
                    W1[:, tb0 + 1:tb1 + 1], a2s[:, tb0:tb1], pcsb[:, 0:blk],
                    W1[:, tb0:tb0 + 1], Alu.add, Alu.mult,
                )
                nc.vector.tensor_tensor_scan(
                    W2[:, tb0 + 1:tb1 + 1], W1[:, tb0:tb1], pcsb[:, 0:blk],
                    W2[:, tb0:tb0 + 1], Alu.add, Alu.mult,
                )
                # u2/20 = W1 + W2 -> layer-2 column of Up, lagged by LAG
                nc.gpsimd.tensor_tensor(
                    Up[0:84, 32, tb0 + LAG:tb1 + LAG],
                    W1[:, tb0 + 1:tb1 + 1], W2[:, tb0 + 1:tb1 + 1], Alu.add,
                )

            # ---- fused scan: layer-1 at step tau, layer-2 at tau-LAG ----
            MW = M.ap[0][0]          # mega-tile row stride (elements)
            MOFF = M.offset

            def m_ap(off, dims, parts=PMAX):
                return bass.AP(M.tensor, MOFF + off, [[MW, parts]] + dims)

            def emit_uchunk(g0):
                if g0 >= T:
                    return
                g1 = min(g0 + UCHUNK, T)
                w = g1 - g0
                # Up[p, j, t] = A1[p, j] * c20[t] — broadcast outer product
                nc.gpsimd.tensor_tensor(
                    Up[:, 0:32, g0:g1],
                    A1[:].unsqueeze(2).broadcast_to([PMAX, 32, w]),
                    c20sb[:, g0:g1].unsqueeze(1).broadcast_to([PMAX, 32, w]),
                    Alu.mult,
                )

            # geometric lead-in chunks: op3(t) never waits on a chunk much
            # bigger than its own step budget
            lb = 0
            for w in (1, 1, 2, 4, 8, 8):
                nc.gpsimd.tensor_tensor(
                    Up[:, 0:32, lb:lb + w],
                    A1[:].unsqueeze(2).broadcast_to([PMAX, 32, w]),
                    c20sb[:, lb:lb + w].unsqueeze(1).broadcast_to(
                        [PMAX, 32, w]),
                    Alu.mult,
                )
                lb += w

            for tau in range(TF):
                if tau < T:
                    pp = m_ap(SW, [[1, NCOL]])
                    rr = m_ap(SW + NCOL, [[1, NCOL]])
                    w0, off = NCOL, 0
                else:
                    # tail: layer-1 finished, only column 32 is live
                    pp = m_ap(SW + 32, [[1, 1]])
                    rr = m_ap(SW + NCOL + 32, [[1, 1]])
                    w0, off = 1, 32
                # P = q*P + s_{tau-1}
                nc.vector.scalar_tensor_tensor(
                    pp, pp, q, m_ap(tau * NCOL + off, [[1, w0]]),
                    Alu.mult, Alu.add,
                )
                # R = q*R + P
                nc.vector.scalar_tensor_tensor(
                    rr, rr, q, pp, Alu.mult, Alu.add,
                )
                # s_{tau} = (R + 0.5) <= u/20
                nc.vector.scalar_tensor_tensor(
                    m_ap((tau + 1) * NCOL + off, [[1, w0]]),
                    rr, 0.5,
                    bass.AP(Up.tensor, Up.offset + off * TF + tau,
                            [[UW, PMAX], [TF, w0]]),
                    Alu.add, Alu.is_le,
                )

                if tau == OSPLIT - 1 + LAG and tau >= T:
                    nc.scalar.activation(
                        ostage[:, 0:OSPLIT],
                        m_ap((LAG + 1) * NCOL + 32, [[NCOL, OSPLIT]],
                             parts=84),
                        Act.Copy,
                    )
                    nc.sync.dma_start(out_d[:, 0:OSPLIT], ostage[:, 0:OSPLIT])
                if tau < T:
                    # PE fc2 block launch (waits on spike history via sems)
                    if tau in blk_at:
                        bi, tb0, tb1 = blk_at[tau]
                        blk = tb1 - tb0
                        a2ps = ps2.tile([NO2, B, NB], f32, tag="a2ps",
                                        name="a2ps")
                        a2ps_t[bi] = a2ps
                        for c in range(4):
                            nc.tensor.matmul(
                                a2ps[:, 0:blk, :],
                                w2sb[:, c, :],
                                m_ap((tb0 + 1) * NCOL + c * NB,
                                     [[NCOL, blk], [1, NB]]),
                                start=(c == 0), stop=(c == 3),
                            )
                    # threshold chunks stay 2 ahead of the scan
                    if tau % UCHUNK == 0 and tau + 2 * UCHUNK < T + UCHUNK:
                        emit_uchunk(tau + 2 * UCHUNK)
                    if tau == OSPLIT - 1 + LAG:
                        # early output half overlaps the remaining tail
                        nc.scalar.activation(
                            ostage[:, 0:OSPLIT],
                            m_ap((LAG + 1) * NCOL + 32, [[NCOL, OSPLIT]],
                                 parts=84),
                            Act.Copy,
                        )
                        nc.sync.dma_start(out_d[:, 0:OSPLIT],
                                          ostage[:, 0:OSPLIT])
                    for bi, which in evac_at.get(tau, []):
                        emit_evac(bi, which)
                        if which == 1:
                            emit_restack(bi)
                    # DVE stream: delayed PSP scans
                    for bi in wscan_at.get(tau, []):
                        emit_wscan(bi)

            # ---- output: remaining layer-2 spikes ----
            nc.scalar.activation(
                ostage[:, OSPLIT:T],
                m_ap((OSPLIT + LAG + 1) * NCOL + 32, [[NCOL, T - OSPLIT]],
                     parts=84),
                Act.Copy,
            )
            nc.sync.dma_start(out_d[:, OSPLIT:T], ostage[:, OSPLIT:T])

    nc.compile()
    return nc


def _host_inputs(input, w1, w2):
    f32 = np.float32
    q, p, k2, c20 = _consts()
    flat = np.ascontiguousarray(input.reshape(64, -1).astype(f32))
    # fTp[p, k, n] = flat[n, k*128+p]
    fTp = np.ascontiguousarray(
        flat.T.reshape(KT, 128, 64).transpose(1, 0, 2))  # (128, KT, 64)
    # w1p[p, k*410+o] = w1[o, k*128+p]
    w1T = w1.astype(f32).T.reshape(KT, 128, NO1)          # (k, p, o)
    w1p = np.ascontiguousarray(
        w1T.transpose(1, 0, 2).reshape(128, KT * NO1))
    w2p = np.zeros((PMAX, 4, NO2), f32)
    for c in range(4):
        w2p[0:MC[c], c, :] = w2.astype(f32)[:, OFF[c]:OFF[c] + MC[c]].T
    pconst = np.full((84, B), p, f32)
    eye8 = np.eye(NB, dtype=f32)
    sel32 = np.zeros((NO2, 3, 32), f32)
    for j in range(3):
        for o2 in range(NO2):
            sel32[o2, j, 10 * j + o2] = 1.0
    c20rep = np.broadcast_to(c20, (PMAX, T)).copy()
    return fTp, w1p, w2p, pconst, eye8, sel32, c20rep


def kernel(input, w1, w2):
    from concourse.bass_utils import run_bass_kernel_spmd

    if "nc" not in _CACHE:
        _CACHE["nc"] = build()
    nc = _CACHE["nc"]

    fTp, w1p, w2p, pconst, eye8, sel32, c20rep = _host_inputs(input, w1, w2)
    in_maps = []
    for core in range(8):
        fTc = fTp[:, :, core * NB:(core + 1) * NB].reshape(128, KT * NB)
        in_maps.append({
            "fTp": np.ascontiguousarray(fTc),
            "w1p": w1p,
            "w2p": w2p,
            "pconst": pconst,
            "eye8": eye8,
            "sel32": sel32,
            "c20rep": c20rep,
        })
    res = run_bass_kernel_spmd(nc, in_maps, core_ids=list(range(8)))
    # row r = 32*g + 10*j + o2 holds batch n = 3*g + j
    rows = np.array([32 * (n // 3) + 10 * (n % 3) + np.arange(NO2)
                     for n in range(NB)])          # (8, 10)
    full = np.zeros((64, NO2, T), np.float32)
    for core in range(8):
        o = res.results[core]["out"]               # (84, T)
        full[core * NB:(core + 1) * NB] = o[rows]
    return full
